# revision 15
# baseline (speedup 1.0000x reference)
"""Trainium2 Bass kernel for nn_Block_21792664060277 (gnn_message_passing).

Strategy (8 NeuronCores, SPMD):
  - Destination-node sharding: 256 graph nodes -> 8 cores x 32 slots,
    greedy-balanced by in-degree; scatter-mean becomes core-local.
  - Kernel 1 (per core, own 32 nodes): LayerNorm (stats via bn_stats,
    weights folded), qkv projection producing h-major transposed q/k
    tables [64, (h, tok)] and token-major v tables.
  - Host: gathers k tables into per-edge order, appends an augmented
    contract row (beta row: 0 for real edges, -1e5 for padding edges so
    exp underflows to exactly 0) and a fake m-column per head whose V
    row is (0..0, eps) so padded rowsums are eps (no NaN); scales the
    V ones-column by deg so the reciprocal of the rowsum directly gives
    the scatter-mean weight.
  - Kernel 2 (per core): per-edge QK^T (contract 65) -> ONE exp
    activation per edge over all heads/chunks -> AV with rowsum column
    -> reciprocal + 6 scalar_tensor_tensor accumulates; then proj
    (+masked bias), residual, LN2 + MLP (gelu), final residual.
"""
import sys

if "/opt/trn_rl_repo" not in sys.path:
    sys.path.insert(0, "/opt/trn_rl_repo")


def _ensure_ntff_hook():
    """Register the axon NTFF profiling hook if the antenv.axon_hooks
    shim module is absent (dropped from some images); without it
    run_bass_kernel_spmd(trace=True) silently skips profiling."""
    try:
        import antenv.axon_hooks  # noqa: F401
        return
    except ImportError:
        pass
    try:
        import types
        import antenv
        mod = types.ModuleType("antenv.axon_hooks")
        _h = {"hook": None}

        def set_axon_ntff_profile_hook(hook):
            _h["hook"] = hook

        def get_axon_ntff_profile_hook():
            return _h["hook"]

        mod.set_axon_ntff_profile_hook = set_axon_ntff_profile_hook
        mod.get_axon_ntff_profile_hook = get_axon_ntff_profile_hook
        sys.modules["antenv.axon_hooks"] = mod
        antenv.axon_hooks = mod
        from trn_agent_boot.trn_boot import _ntff_profile_via_ctypes
        hook = _ntff_profile_via_ctypes("/opt/axon/libaxon_pjrt.so")
        if hook is not None:
            set_axon_ntff_profile_hook(hook)
    except Exception:
        pass


_ensure_ntff_hook()

import numpy as np
import ml_dtypes

import concourse.bass as bass  # noqa: F401
import concourse.bacc as bacc
import concourse.mybir as mybir
import concourse.tile as tile
from concourse import bass_utils
from concourse.masks import make_identity

BF16 = mybir.dt.bfloat16
F32 = mybir.dt.float32

Bn, N, C = 256, 197, 192
H, HD = 3, 64
HID = 768
NCORES = 8
SLOTS = 32
TOK = SLOTS * N          # 6304
EPS = 1e-5
SCALE = HD ** -0.5
BETA = -1.0e5            # pad-edge logit offset (exp underflows to 0)
FAKE_EPS = 1e-30         # fake-column rowsum epsilon
KC = 256                 # per-head k-table cols (197 real + fake + pad to 128x2)
KCOLS = H * KC           # 768: per-edge k table cols (h-major)

MC = [(0, 128), (128, 69)]      # token chunks within a node (197 = 128+69)

TRACE = False
LAST_EXEC_NS = None

bf = ml_dtypes.bfloat16


def _bf(a):
    return np.ascontiguousarray(np.asarray(a, np.float32)).astype(bf)


def _plan(edge):
    """Node->core assignment balanced by degree + shared degree profile."""
    src, dst = np.asarray(edge[0]), np.asarray(edge[1])
    deg = np.bincount(dst, minlength=Bn)
    order = np.argsort(-deg, kind="stable")
    cores = [[] for _ in range(NCORES)]
    loads = np.zeros(NCORES, np.int64)
    for n in order:
        free = [c for c in range(NCORES) if len(cores[c]) < SLOTS]
        c = min(free, key=lambda c: (loads[c], c))
        cores[c].append(int(n))
        loads[c] += deg[n]
    prof = np.zeros(SLOTS, np.int64)
    for c in range(NCORES):
        ds = np.array([deg[n] for n in cores[c]])
        prof = np.maximum(prof, ds)
    prof = prof.astype(int)
    e_pad = max(int(prof.sum()), 1)
    starts = np.concatenate([[0], np.cumsum(prof)]).astype(int)
    by_dst = [[] for _ in range(Bn)]
    for e in range(src.shape[0]):
        by_dst[int(dst[e])].append(int(src[e]))
    sched_src = np.zeros((NCORES, e_pad), np.int64)
    is_pad = np.ones((NCORES, e_pad), bool)
    degs = np.zeros((NCORES, SLOTS), np.int64)
    mask = np.zeros((NCORES, SLOTS), np.float32)
    for c in range(NCORES):
        for s in range(SLOTS):
            node = cores[c][s]
            lst = by_dst[node]
            degs[c, s] = len(lst)
            mask[c, s] = 1.0 if lst else 0.0
            for j in range(prof[s]):
                p = starts[s] + j
                if j < len(lst):
                    sched_src[c, p] = lst[j]
                    is_pad[c, p] = False
    return cores, prof, e_pad, starts, sched_src, is_pad, degs, mask


def _build_kernel1():
    nc = bacc.Bacc("TRN2", target_bir_lowering=False, debug=False,
                   num_devices=NCORES)
    x_in = nc.dram_tensor("x_own", [TOK, C], F32, kind="ExternalInput")
    wqk = nc.dram_tensor("wqkT_aug", [C + 1, 2 * C], BF16, kind="ExternalInput")
    wv = nc.dram_tensor("wvT_aug", [C + 1, H * 65], BF16, kind="ExternalInput")
    ones_in = nc.dram_tensor("ones_row", [1, TOK], BF16, kind="ExternalInput")
    qt_out = nc.dram_tensor("qT_tab", [HD, H * TOK], BF16, kind="ExternalOutput")
    kt_out = nc.dram_tensor("kT_tab", [HD, H * TOK], BF16, kind="ExternalOutput")
    v_hi_out = nc.dram_tensor("v_hi", [128, SLOTS * 195], BF16, kind="ExternalOutput")
    v_lo_out = nc.dram_tensor("v_lo", [69, SLOTS * 195], BF16, kind="ExternalOutput")

    NT = (TOK + 127) // 128      # 50 token tiles (49 full + 32)

    with tile.TileContext(nc) as tc:
        with tc.tile_pool(name="cst", bufs=1) as cst:
            ident = cst.tile([128, 128], BF16)
            make_identity(nc, ident[:])
            eps_t = cst.tile([128, 1], F32)
            nc.vector.memset(eps_t[:], EPS)
            wqk_a = cst.tile([128, 2 * C], BF16)
            wqk_b = cst.tile([65, 2 * C], BF16)
            nc.sync.dma_start(wqk_a[:], wqk[0:128, :])
            nc.sync.dma_start(wqk_b[:], wqk[128:193, :])
            wv_a = cst.tile([128, H * 65], BF16)
            wv_b = cst.tile([65, H * 65], BF16)
            nc.sync.dma_start(wv_a[:], wv[0:128, :])
            nc.sync.dma_start(wv_b[:], wv[128:193, :])
            x_res = cst.tile([128, NT * C], F32)
            stats = cst.tile([128, NT, 2], F32)
            sd = cst.tile([128, NT, 1], F32)
            istd = cst.tile([128, NT, 1], F32)
            xhT_a = cst.tile([128, TOK], BF16)
            xhT_b = cst.tile([65, TOK], BF16)
            nc.sync.dma_start(xhT_b[64:65, :], ones_in[:])

            TGRP = 4
            with tc.tile_pool(name="pa", bufs=3) as sba, \
                 tc.tile_pool(name="pbt", bufs=1, space="PSUM") as pbt, \
                 tc.tile_pool(name="pc", bufs=3) as sbc, \
                 tc.tile_pool(name="pcp", bufs=2, space="PSUM") as pcp, \
                 tc.tile_pool(name="pd", bufs=3) as sbd, \
                 tc.tile_pool(name="pdp", bufs=2, space="PSUM") as pdp:
                # ---- pass A: load x (batched), LN stats ----
                for g0 in range(0, TOK, 512):
                    gl = min(512, TOK - g0)
                    if gl == 512:
                        nc.sync.dma_start(
                            x_res[:, g0 // 128 * C:(g0 // 128 + 4) * C]
                            .rearrange("p (i c) -> p i c", c=C),
                            x_in[g0:g0 + 512, :].rearrange("(i p) c -> p i c", p=128))
                    else:
                        for t0 in range(g0, TOK, 128):
                            tl = min(128, TOK - t0)
                            nc.sync.dma_start(
                                x_res[0:tl, t0 // 128 * C:(t0 // 128 + 1) * C],
                                x_in[t0:t0 + tl, :])
                for t in range(NT):
                    tl = min(128, TOK - t * 128)
                    st6 = sba.tile([128, 6], F32, tag="st6")
                    nc.vector.bn_stats(st6[0:tl, :], x_res[0:tl, t * C:(t + 1) * C])
                    nc.vector.bn_aggr(stats[0:tl, t, :], st6[0:tl, :])
                nc.scalar.activation(sd[:], stats[:, :, 1:2],
                                     mybir.ActivationFunctionType.Sqrt,
                                     bias=eps_t[:])
                nc.vector.reciprocal(istd[:], sd[:])

                # ---- pass B: xhat + transpose ----
                for t in range(NT):
                    g0 = t * 128
                    tl = min(128, TOK - g0)
                    xh = sba.tile([128, C], BF16, tag="xh")
                    nc.vector.tensor_scalar(xh[0:tl, :], x_res[0:tl, t * C:(t + 1) * C],
                                            stats[0:tl, t, 0:1], istd[0:tl, t, :],
                                            mybir.AluOpType.subtract,
                                            mybir.AluOpType.mult)
                    tp0 = pbt.tile([128, 128], BF16, tag="tp0")
                    tp1 = pbt.tile([64, 128], BF16, tag="tp1")
                    nc.tensor.transpose(tp0[:, 0:tl], xh[0:tl, 0:128], ident[0:tl, 0:tl])
                    nc.tensor.transpose(tp1[:, 0:tl], xh[0:tl, 128:192], ident[0:tl, 0:tl])
                    nc.vector.tensor_copy(out=xhT_a[:, g0:g0 + tl], in_=tp0[:, 0:tl])
                    nc.scalar.copy(out=xhT_b[0:64, g0:g0 + tl], in_=tp1[:, 0:tl])

                # ---- pass C: q/k projections (h-major tables) ----
                for gg in range(0, NT, TGRP):
                    gn = min(TGRP, NT - gg)
                    for cc in range(3):
                        qkp = pcp.tile([128, TGRP * 128], F32, tag="qkp",
                                       name=f"qkp_{gg}_{cc}")
                        for tt in range(gn):
                            g0 = (gg + tt) * 128
                            tl = min(128, TOK - g0)
                            nc.tensor.matmul(qkp[:, tt * 128:tt * 128 + tl],
                                             wqk_a[:, cc * 128:(cc + 1) * 128],
                                             xhT_a[:, g0:g0 + tl],
                                             start=True, stop=False)
                            nc.tensor.matmul(qkp[:, tt * 128:tt * 128 + tl],
                                             wqk_b[:, cc * 128:(cc + 1) * 128],
                                             xhT_b[:, g0:g0 + tl],
                                             start=False, stop=True)
                        g0 = gg * 128
                        glen = min(TGRP * 128, TOK - g0)
                        for half in range(2):
                            gidx = cc * 2 + half
                            if gidx < 3:
                                dstt, hh = qt_out, gidx
                            else:
                                dstt, hh = kt_out, gidx - 3
                            stg = sbc.tile([64, TGRP * 128], BF16, tag="stg")
                            if half == 0:
                                nc.vector.tensor_copy(out=stg[:, 0:glen],
                                                      in_=qkp[0:64, 0:glen])
                            else:
                                nc.scalar.copy(out=stg[:, 0:glen],
                                               in_=qkp[64:128, 0:glen])
                            nc.sync.dma_start(
                                dstt[:, hh * TOK + g0: hh * TOK + g0 + glen],
                                stg[:, 0:glen])

                # ---- pass D: v projection (token-major per slot) ----
                for s in range(SLOTS):
                    for mi, (m0, ml) in enumerate(MC):
                        r0 = s * N + m0
                        vp = pdp.tile([128, H * 65], F32, tag="vp")
                        nc.tensor.matmul(vp[0:ml, :], xhT_a[:, r0:r0 + ml], wv_a[:],
                                         start=True, stop=False)
                        nc.tensor.matmul(vp[0:ml, :], xhT_b[:, r0:r0 + ml], wv_b[:],
                                         start=False, stop=True)
                        vsb = sbd.tile([128, H * 65], BF16, tag="vsb")
                        if mi == 0:
                            nc.vector.tensor_copy(out=vsb[0:ml, :], in_=vp[0:ml, :])
                        else:
                            nc.scalar.copy(out=vsb[0:ml, :], in_=vp[0:ml, :])
                        dstt = v_hi_out if mi == 0 else v_lo_out
                        nc.sync.dma_start(dstt[0:ml, s * 195:(s + 1) * 195],
                                          vsb[0:ml, :])
    nc.compile()
    return nc


def _build_kernel2(prof, e_pad):
    starts = np.concatenate([[0], np.cumsum(prof)]).astype(int)
    nc = bacc.Bacc("TRN2", target_bir_lowering=False, debug=False,
                   num_devices=NCORES)
    x_in = nc.dram_tensor("x_own", [TOK + 64, C], F32, kind="ExternalInput")
    qt_in = nc.dram_tensor("qT_aug", [HD + 1, H * TOK], BF16, kind="ExternalInput")
    v_hi_in = nc.dram_tensor("v_hi_aug", [128, SLOTS * 195], BF16, kind="ExternalInput")
    v_lo_in = nc.dram_tensor("v_lo_aug", [70, SLOTS * 195], BF16, kind="ExternalInput")
    kte_in = nc.dram_tensor("kT_edges", [e_pad * (HD + 1), KCOLS], BF16,
                            kind="ExternalInput")
    mrow_in = nc.dram_tensor("maskrow", [1, TOK], BF16, kind="ExternalInput")
    ones_in = nc.dram_tensor("ones_row", [1, TOK], BF16, kind="ExternalInput")
    pw_in = nc.dram_tensor("projWT", [C, C], BF16, kind="ExternalInput")
    pb_in = nc.dram_tensor("projb", [1, C], BF16, kind="ExternalInput")
    w1_in = nc.dram_tensor("w1T_aug", [C + 1, HID], BF16, kind="ExternalInput")
    w2_in = nc.dram_tensor("w2T_aug", [HID + 1, C], BF16, kind="ExternalInput")
    out = nc.dram_tensor("out_own", [TOK, C], F32, kind="ExternalOutput")

    NHC = [(0, 128), (128, 69)]   # n-chunks (dest tokens) within a node
    NT = (TOK + 127) // 128

    with tile.TileContext(nc) as tc:
        with tc.tile_pool(name="cst", bufs=1) as cst, \
             tc.tile_pool(name="dram", bufs=1, space="DRAM") as dpool:
            x2_dram = dpool.tile([TOK + 96, C], F32)
            ident_bf = cst.tile([128, 128], BF16)
            make_identity(nc, ident_bf[:])
            eps_t = cst.tile([128, 1], F32)
            nc.vector.memset(eps_t[:], EPS)
            qt_sb = cst.tile([HD + 1, H * TOK], BF16)
            nc.sync.dma_start(qt_sb[:], qt_in[:])
            v_hi = cst.tile([128, SLOTS * 195], BF16)
            v_lo = cst.tile([70, SLOTS * 195], BF16)
            nc.sync.dma_start(v_hi[:], v_hi_in[:])
            nc.sync.dma_start(v_lo[:], v_lo_in[:])
            mrow = cst.tile([1, TOK], BF16)
            nc.sync.dma_start(mrow[:], mrow_in[:])
            pw_a = cst.tile([128, C], BF16)
            pw_b = cst.tile([64, C], BF16)
            nc.sync.dma_start(pw_a[:], pw_in[0:128, :])
            nc.sync.dma_start(pw_b[:], pw_in[128:192, :])
            pb_sb = cst.tile([1, C], BF16)
            nc.sync.dma_start(pb_sb[:], pb_in[:])
            acc = cst.tile([128, SLOTS * 2 * C], F32)
            nc.vector.memset(acc[:], 0.0)
            stats = cst.tile([128, 2 * SLOTS, 2], F32)
            sd = cst.tile([128, 2 * SLOTS, 1], F32)
            istd = cst.tile([128, 2 * SLOTS, 1], F32)
            xh2T_a = cst.tile([128, TOK], BF16)
            xh2T_b = cst.tile([65, TOK], BF16)
            nc.sync.dma_start(xh2T_b[64:65, :], ones_in[:])

            # ---------------- phase B: per-edge attention ----------------
            edges = [(s, starts[s] + j) for s in range(SLOTS)
                     for j in range(int(prof[s]))]
            with tc.tile_pool(name="pb_sb", bufs=3) as sbb, \
                 tc.tile_pool(name="ps_s", bufs=2, space="PSUM") as ps_s, \
                 tc.tile_pool(name="ps_m", bufs=2, space="PSUM") as ps_m:

                kst_cur = [None]

                def emit_qk_exp(s, ep):
                    if ep % 2 == 0:
                        kst = sbb.tile([HD + 1, 2, KCOLS], BF16, tag="kst",
                                       name=f"kst_{ep}")
                        npair = min(2, e_pad - ep)
                        nc.sync.dma_start(
                            kst[:, 0:npair, :],
                            kte_in[ep * 65:(ep + npair) * 65, :]
                            .rearrange("(e r) c -> r e c", e=npair))
                        kst_cur[0] = kst
                    kst = kst_cur[0][:, ep % 2, :]
                    S = ps_s.tile([128, 3 * 512], F32, tag="S", name=f"S_{ep}")
                    for h in range(H):
                        nc.tensor.matmul(
                            S[0:128, h * 512: h * 512 + N],
                            kst[:, h * KC: h * KC + 128],
                            qt_sb[:, h * TOK + s * N: h * TOK + (s + 1) * N],
                            start=True, stop=True)
                        nc.tensor.matmul(
                            S[0:128, h * 512 + N: h * 512 + 2 * N],
                            kst[:, h * KC + 128: h * KC + 256],
                            qt_sb[:, h * TOK + s * N: h * TOK + (s + 1) * N],
                            start=True, stop=True)
                    E = sbb.tile([128, 3, 456], BF16, tag="E", name=f"E_{ep}")
                    nc.scalar.activation(
                        E[:, :, 0:2 * N],
                        S[:].rearrange("p (h c) -> p h c", h=3)[:, :, 0:2 * N],
                        mybir.ActivationFunctionType.Exp, scale=SCALE)
                    return S, E

                def emit_av_norm(s, ep, E):
                    msg = ps_m.tile([128, 2 * 195], F32, tag="msg",
                                    name=f"msg_{ep}")
                    for ni, (n0, nl) in enumerate(NHC):
                        for h in range(H):
                            nc.tensor.matmul(
                                msg[0:128, ni * 195 + h * 65: ni * 195 + (h + 1) * 65],
                                E[0:128, h, n0: n0 + 128],
                                v_hi[0:128, s * 195 + h * 65: s * 195 + (h + 1) * 65],
                                start=True, stop=False)
                            nc.tensor.matmul(
                                msg[0:128, ni * 195 + h * 65: ni * 195 + (h + 1) * 65],
                                E[0:70, h, N + n0: N + n0 + 128],
                                v_lo[0:70, s * 195 + h * 65: s * 195 + (h + 1) * 65],
                                start=False, stop=True)
                    rec = sbb.tile([128, 2, 3, 1], F32, tag="rec", name=f"rec_{ep}")
                    nc.vector.reciprocal(
                        rec[:],
                        msg[:].rearrange("p (i h c) -> p i h c", i=2, c=65)[:, :, :, 64:65])
                    for ni, (n0, nl) in enumerate(NHC):
                        for h in range(H):
                            a_sl = acc[0:nl,
                                       s * 2 * C + ni * C + h * 64:
                                       s * 2 * C + ni * C + (h + 1) * 64]
                            nc.vector.scalar_tensor_tensor(
                                out=a_sl,
                                in0=msg[0:nl, ni * 195 + h * 65: ni * 195 + h * 65 + 64],
                                scalar=rec[0:nl, ni, h, :],
                                in1=a_sl,
                                op0=mybir.AluOpType.mult,
                                op1=mybir.AluOpType.add)

                prev = None
                for (s, ep) in edges:
                    S, E = emit_qk_exp(s, ep)
                    if prev is not None:
                        emit_av_norm(*prev)
                    prev = (s, ep, E)
                emit_av_norm(*prev)

            # ---------------- phase C: proj + residual + LN2 stats ----------------
            with tc.tile_pool(name="pc_sb", bufs=3) as sbc, \
                 tc.tile_pool(name="ps_t", bufs=2, space="PSUM") as ps_t, \
                 tc.tile_pool(name="ps_c", bufs=2, space="PSUM") as ps_c:
                for s in range(SLOTS):
                    acc_bf = sbc.tile([128, 2 * C], BF16, tag="accbf")
                    nc.vector.tensor_copy(out=acc_bf[:],
                                          in_=acc[:, s * 2 * C:(s + 1) * 2 * C])
                    xt2 = sbc.tile([128, 2, C], F32, tag="xt2")
                    nc.sync.dma_start(
                        xt2[:],
                        x_in[s * N: s * N + 256, :]
                        .rearrange("(i p) c -> p i c", p=128))
                    agT_a = sbc.tile([128, N], BF16, tag="agTa")
                    agT_b = sbc.tile([64, N], BF16, tag="agTb")
                    for ni, (n0, nl) in enumerate(NHC):
                        tp0 = ps_t.tile([128, 128], BF16, tag="tp0")
                        tp1 = ps_t.tile([64, 128], BF16, tag="tp1")
                        nc.tensor.transpose(tp0[:, 0:nl], acc_bf[0:nl, ni * C:ni * C + 128],
                                            ident_bf[0:nl, 0:nl])
                        nc.tensor.transpose(tp1[:, 0:nl], acc_bf[0:nl, ni * C + 128:ni * C + 192],
                                            ident_bf[0:nl, 0:nl])
                        nc.vector.tensor_copy(out=agT_a[:, n0:n0 + nl], in_=tp0[:, 0:nl])
                        nc.scalar.copy(out=agT_b[:, n0:n0 + nl], in_=tp1[:, 0:nl])
                    for ni, (n0, nl) in enumerate(NHC):
                        yp = ps_c.tile([128, C], F32, tag="yp")
                        nc.tensor.matmul(yp[0:nl, :], agT_a[:, n0:n0 + nl], pw_a[:],
                                         start=True, stop=False)
                        nc.tensor.matmul(yp[0:nl, :], agT_b[:, n0:n0 + nl], pw_b[:],
                                         start=False, stop=False)
                        nc.tensor.matmul(yp[0:nl, :],
                                         mrow[0:1, s * N + n0: s * N + n0 + nl],
                                         pb_sb[:], start=False, stop=True)
                        r0 = s * N + n0
                        x2t = sbc.tile([128, C], F32, tag="x2t")
                        nc.vector.tensor_tensor(out=x2t[0:nl, :], in0=yp[0:nl, :],
                                                in1=xt2[0:nl, ni, :],
                                                op=mybir.AluOpType.add)
                        nc.sync.dma_start(x2_dram[r0:r0 + nl, :], x2t[0:nl, :])
                        st6 = sbc.tile([128, 6], F32, tag="st6")
                        nc.vector.bn_stats(st6[0:nl, :], x2t[0:nl, :])
                        nc.vector.bn_aggr(stats[0:nl, s * 2 + ni, :], st6[0:nl, :])

            nc.scalar.activation(sd[:], stats[:, :, 1:2],
                                 mybir.ActivationFunctionType.Sqrt,
                                 bias=eps_t[:])
            nc.vector.reciprocal(istd[:], sd[:])

            # ---------------- phase C2: xh2T build ----------------
            with tc.tile_pool(name="c2_sb", bufs=3) as sb2, \
                 tc.tile_pool(name="c2_ps", bufs=2, space="PSUM") as ps2:
                for s in range(SLOTS):
                    for ni, (n0, nl) in enumerate(NHC):
                        r0 = s * N + n0
                        xt2 = sb2.tile([128, C], F32, tag="xt2")
                        nc.sync.dma_start(xt2[0:nl, :], x2_dram[r0:r0 + nl, :])
                        xh2 = sb2.tile([128, C], BF16, tag="xh2")
                        nc.vector.tensor_scalar(xh2[0:nl, :], xt2[0:nl, :],
                                                stats[0:nl, s * 2 + ni, 0:1],
                                                istd[0:nl, s * 2 + ni, :],
                                                mybir.AluOpType.subtract,
                                                mybir.AluOpType.mult)
                        tp0 = ps2.tile([128, 128], BF16, tag="tp0")
                        tp1 = ps2.tile([64, 128], BF16, tag="tp1")
                        nc.tensor.transpose(tp0[:, 0:nl], xh2[0:nl, 0:128],
                                            ident_bf[0:nl, 0:nl])
                        nc.tensor.transpose(tp1[:, 0:nl], xh2[0:nl, 128:192],
                                            ident_bf[0:nl, 0:nl])
                        nc.vector.tensor_copy(out=xh2T_a[:, r0:r0 + nl], in_=tp0[:, 0:nl])
                        nc.scalar.copy(out=xh2T_b[0:64, r0:r0 + nl], in_=tp1[:, 0:nl])

            # ---------------- phase D: MLP ----------------
            with tc.tile_pool(name="pd_cst", bufs=1) as cd, \
                 tc.tile_pool(name="pd_sb", bufs=3) as sbd, \
                 tc.tile_pool(name="ps_d", bufs=2, space="PSUM") as ps_d:
                w1_a = cd.tile([128, HID], BF16)
                w1_b = cd.tile([65, HID], BF16)
                nc.sync.dma_start(w1_a[:], w1_in[0:128, :])
                nc.sync.dma_start(w1_b[:], w1_in[128:193, :])
                w2_t = []
                for hc in range(6):
                    t = cd.tile([128, C], BF16, tag=f"w2_{hc}", name=f"w2_{hc}")
                    nc.sync.dma_start(t[:], w2_in[hc * 128:(hc + 1) * 128, :])
                    w2_t.append(t)
                w2_bias = cd.tile([1, C], BF16)
                nc.sync.dma_start(w2_bias[:], w2_in[HID:HID + 1, :])
                ones_sb = cd.tile([1, 128], BF16)
                nc.vector.memset(ones_sb[:], 1.0)

                BLK = 512
                for b0 in range(0, TOK, BLK):
                    bl = min(BLK, TOK - b0)
                    h1 = [sbd.tile([128, BLK], BF16, tag=f"h1_{hc}", name=f"h1_{hc}_{b0}")
                          for hc in range(6)]
                    for hc in range(6):
                        hp = ps_d.tile([128, BLK], F32, tag="hp", name=f"hp_{hc}_{b0}")
                        nc.tensor.matmul(hp[:, 0:bl], w1_a[:, hc * 128:(hc + 1) * 128],
                                         xh2T_a[:, b0:b0 + bl], start=True, stop=False)
                        nc.tensor.matmul(hp[:, 0:bl], w1_b[:, hc * 128:(hc + 1) * 128],
                                         xh2T_b[:, b0:b0 + bl], start=False, stop=True)
                        nc.scalar.activation(h1[hc][:, 0:bl], hp[:, 0:bl],
                                             mybir.ActivationFunctionType.Gelu)
                    for u0 in range(0, bl, 256):
                        gu = b0 + u0
                        x2t = sbd.tile([128, 2, C], F32, tag="x2t",
                                       name=f"x2t_{gu}")
                        nc.sync.dma_start(
                            x2t[:],
                            x2_dram[gu:gu + 256, :]
                            .rearrange("(i p) c -> p i c", p=128))
                        ot = sbd.tile([128, 2, C], F32, tag="ot", name=f"ot_{gu}")
                        for v in range(2):
                            t0 = u0 + v * 128
                            if t0 >= bl:
                                continue
                            tl = min(128, bl - t0)
                            op = ps_d.tile([128, C], F32, tag="op",
                                           name=f"op_{b0}_{t0}")
                            for hc in range(6):
                                nc.tensor.matmul(op[0:tl, :], h1[hc][:, t0:t0 + tl],
                                                 w2_t[hc][:], start=(hc == 0),
                                                 stop=False)
                            nc.tensor.matmul(op[0:tl, :],
                                             ones_sb[0:1, 0:tl],
                                             w2_bias[:], start=False, stop=True)
                            nc.vector.tensor_tensor(out=ot[0:tl, v, :],
                                                    in0=op[0:tl, :],
                                                    in1=x2t[0:tl, v, :],
                                                    op=mybir.AluOpType.add)
                        rem = min(256, TOK - gu)
                        if rem == 256:
                            nc.sync.dma_start(
                                out[gu:gu + 256, :]
                                .rearrange("(i p) c -> p i c", p=128),
                                ot[:])
                        else:
                            nc.sync.dma_start(out[gu:gu + 128, :], ot[:, 0, :])
                            if rem > 128:
                                nc.sync.dma_start(out[gu + 128:gu + rem, :],
                                                  ot[0:rem - 128, 1, :])
    nc.compile()
    return nc


def kernel(x, egde, norm1_g, norm1_b, qkv_w, proj_w, proj_b,
           norm2_g, norm2_b, fc1_w, fc1_b, fc2_w, fc2_b):
    x = np.asarray(x, np.float32)
    edge = np.asarray(egde)
    g1 = np.asarray(norm1_g, np.float32)
    b1 = np.asarray(norm1_b, np.float32)
    qkv_w = np.asarray(qkv_w, np.float32)

    cores, prof, e_pad, starts, sched_src, is_pad, degs, mask = _plan(edge)

    # ---- kernel 1 host prep ----
    wqk = (qkv_w[0:2 * C, :] * g1[None, :]).T             # [C, 384]
    bqk = qkv_w[0:2 * C, :] @ b1
    wqkT_aug = _bf(np.concatenate([wqk, bqk[None, :]], 0))
    # v weights rearranged to (h, 65) with zero rowsum columns
    wv = (qkv_w[2 * C:3 * C, :] * g1[None, :]).T          # [C, 192]
    bv = qkv_w[2 * C:3 * C, :] @ b1                       # [192]
    wv_aug = np.zeros((C + 1, H * 65), np.float32)
    for h in range(H):
        wv_aug[0:C, h * 65:h * 65 + 64] = wv[:, h * 64:(h + 1) * 64]
        wv_aug[C, h * 65:h * 65 + 64] = bv[h * 64:(h + 1) * 64]
    wvT_aug = _bf(wv_aug)
    ones_row = _bf(np.ones((1, TOK), np.float32))

    x_own = np.stack([x[cores[c]].reshape(TOK, C) for c in range(NCORES)])

    nc1 = _build_kernel1()
    in_maps1 = [{"x_own": np.ascontiguousarray(x_own[c]),
                 "wqkT_aug": wqkT_aug, "wvT_aug": wvT_aug,
                 "ones_row": ones_row}
                for c in range(NCORES)]
    res1 = bass_utils.run_bass_kernel_spmd(nc1, in_maps1, core_ids=list(range(NCORES)),
                                           trace=TRACE)

    # ---- host gather: build per-edge k tables + augmented inputs ----
    # kT_tab: [64, (h, TOK)] -> global [64, H, Bn, N]
    kt_glob = np.zeros((HD, H, Bn, N), bf)
    for c in range(NCORES):
        sh = res1.results[c]["kT_tab"].reshape(HD, H, SLOTS, N)
        for s in range(SLOTS):
            kt_glob[:, :, cores[c][s], :] = sh[:, :, s, :]
    kte = np.zeros((NCORES, e_pad * (HD + 1), KCOLS), bf)
    beta_bf = np.array(BETA, bf)
    for c in range(NCORES):
        gathered = kt_glob[:, :, sched_src[c], :]         # [64, H, e_pad, N]
        blk = np.zeros((e_pad, HD + 1, H, KC), bf)
        blk[:, 0:HD, :, 0:N] = gathered.transpose(2, 0, 1, 3)
        # beta row: -1e5 on real m-cols of pad edges; 0 on fake/pad cols
        blk[is_pad[c], HD, :, 0:N] = beta_bf
        kte[c] = blk.reshape(e_pad * (HD + 1), KCOLS)

    # q tables + ones contract row
    qt_aug = np.zeros((NCORES, HD + 1, H * TOK), bf)
    for c in range(NCORES):
        qt_aug[c, 0:HD] = res1.results[c]["qT_tab"]
        qt_aug[c, HD] = np.array(1.0, bf)

    # v tables: deg-scaled rowsum columns + fake row (eps)
    v_hi_aug = np.zeros((NCORES, 128, SLOTS * 195), bf)
    v_lo_aug = np.zeros((NCORES, 70, SLOTS * 195), bf)
    fake = np.zeros((SLOTS * 195,), np.float32)
    for c in range(NCORES):
        vh = np.asarray(res1.results[c]["v_hi"], bf).copy()
        vl = np.asarray(res1.results[c]["v_lo"], bf).copy()
        for s in range(SLOTS):
            d = float(degs[c, s])
            for h in range(H):
                col = s * 195 + h * 65 + 64
                vh[:, col] = np.array(d, bf)
                vl[:, col] = np.array(d, bf)
                fake[col] = FAKE_EPS
        v_hi_aug[c, :, :] = vh
        v_lo_aug[c, 0:69, :] = vl
        v_lo_aug[c, 69, :] = fake.astype(bf)

    # ---- kernel 2 host prep ----
    g2 = np.asarray(norm2_g, np.float32)
    b2 = np.asarray(norm2_b, np.float32)
    fc1_w = np.asarray(fc1_w, np.float32)
    fc2_w = np.asarray(fc2_w, np.float32)
    w1 = (fc1_w * g2[None, :]).T
    bb1 = fc1_w @ b2 + np.asarray(fc1_b, np.float32)
    w1T_aug = _bf(np.concatenate([w1, bb1[None, :]], 0))
    w2T_aug = _bf(np.concatenate([fc2_w.T, np.asarray(fc2_b, np.float32)[None, :]], 0))
    projWT = _bf(np.asarray(proj_w, np.float32).T)
    projb = _bf(np.asarray(proj_b, np.float32)[None, :])
    maskrow = _bf(np.repeat(mask, N, axis=1)[:, None, :])

    x_own_pad = np.zeros((NCORES, TOK + 64, C), np.float32)
    x_own_pad[:, 0:TOK, :] = x_own

    nc2 = _build_kernel2(prof, e_pad)
    in_maps2 = []
    for c in range(NCORES):
        in_maps2.append({
            "x_own": np.ascontiguousarray(x_own_pad[c]),
            "qT_aug": np.ascontiguousarray(qt_aug[c]),
            "v_hi_aug": np.ascontiguousarray(v_hi_aug[c]),
            "v_lo_aug": np.ascontiguousarray(v_lo_aug[c]),
            "kT_edges": np.ascontiguousarray(kte[c]),
            "maskrow": np.ascontiguousarray(maskrow[c]),
            "ones_row": ones_row,
            "projWT": projWT, "projb": projb,
            "w1T_aug": w1T_aug, "w2T_aug": w2T_aug,
        })
    res2 = bass_utils.run_bass_kernel_spmd(nc2, in_maps2, core_ids=list(range(NCORES)),
                                           trace=TRACE)
    global LAST_EXEC_NS
    LAST_EXEC_NS = [res1.exec_time_ns or 0, res2.exec_time_ns or 0]

    outp = np.zeros((Bn, N, C), np.float32)
    for c in range(NCORES):
        outp[cores[c]] = res2.results[c]["out_own"].reshape(SLOTS, N, C)
    return outp


# revision 17
# speedup vs baseline: 1.1542x; 1.1542x over previous
"""Trainium2 Bass kernel for nn_Block_21792664060277 (gnn_message_passing).

Strategy (8 NeuronCores, SPMD):
  - Destination-node sharding: 256 graph nodes -> 8 cores x 32 slots,
    greedy-balanced by in-degree; scatter-mean becomes core-local.
  - Kernel 1 (per core, own 32 nodes): LayerNorm (stats via bn_stats,
    weights folded), qkv projection producing h-major transposed q/k
    tables [64, (h, tok)] and token-major v tables.
  - Host: gathers k tables into per-edge order, appends an augmented
    contract row (beta row: 0 for real edges, -1e5 for padding edges so
    exp underflows to exactly 0) and a fake m-column per head whose V
    row is (0..0, eps) so padded rowsums are eps (no NaN); scales the
    V ones-column by deg so the reciprocal of the rowsum directly gives
    the scatter-mean weight.
  - Kernel 2 (per core): per-edge QK^T (contract 65) -> ONE exp
    activation per edge over all heads/chunks -> AV with rowsum column
    -> reciprocal + 6 scalar_tensor_tensor accumulates; then proj
    (+masked bias), residual, LN2 + MLP (gelu), final residual.
"""
import sys

if "/opt/trn_rl_repo" not in sys.path:
    sys.path.insert(0, "/opt/trn_rl_repo")


def _ensure_ntff_hook():
    """Register the axon NTFF profiling hook if the antenv.axon_hooks
    shim module is absent (dropped from some images); without it
    run_bass_kernel_spmd(trace=True) silently skips profiling."""
    try:
        import antenv.axon_hooks  # noqa: F401
        return
    except ImportError:
        pass
    try:
        import types
        import antenv
        mod = types.ModuleType("antenv.axon_hooks")
        _h = {"hook": None}

        def set_axon_ntff_profile_hook(hook):
            _h["hook"] = hook

        def get_axon_ntff_profile_hook():
            return _h["hook"]

        mod.set_axon_ntff_profile_hook = set_axon_ntff_profile_hook
        mod.get_axon_ntff_profile_hook = get_axon_ntff_profile_hook
        sys.modules["antenv.axon_hooks"] = mod
        antenv.axon_hooks = mod
        from trn_agent_boot.trn_boot import _ntff_profile_via_ctypes
        hook = _ntff_profile_via_ctypes("/opt/axon/libaxon_pjrt.so")
        if hook is not None:
            set_axon_ntff_profile_hook(hook)
    except Exception:
        pass


_ensure_ntff_hook()

import numpy as np
import ml_dtypes

import concourse.bass as bass  # noqa: F401
import concourse.bacc as bacc
import concourse.mybir as mybir
import concourse.tile as tile
from concourse import bass_utils
from concourse.masks import make_identity

BF16 = mybir.dt.bfloat16
F32 = mybir.dt.float32

Bn, N, C = 256, 197, 192
H, HD = 3, 64
HID = 768
NCORES = 8
SLOTS = 32
TOK = SLOTS * N          # 6304
EPS = 1e-5
SCALE = HD ** -0.5
BETA = -1.0e5            # pad-edge logit offset (exp underflows to 0)
FAKE_EPS = 1e-30         # fake-column rowsum epsilon
KC = 198                 # per-head k-table cols (197 real + 1 fake)
KCOLS = H * KC           # 594: per-edge k table cols (h-major)

MC = [(0, 128), (128, 69)]      # token chunks within a node (197 = 128+69)

TRACE = False
LAST_EXEC_NS = None

bf = ml_dtypes.bfloat16


def _bf(a):
    return np.ascontiguousarray(np.asarray(a, np.float32)).astype(bf)


def _plan(edge):
    """Node->core assignment balanced by degree + shared degree profile."""
    src, dst = np.asarray(edge[0]), np.asarray(edge[1])
    deg = np.bincount(dst, minlength=Bn)
    order = np.argsort(-deg, kind="stable")
    cores = [[] for _ in range(NCORES)]
    loads = np.zeros(NCORES, np.int64)
    for n in order:
        free = [c for c in range(NCORES) if len(cores[c]) < SLOTS]
        c = min(free, key=lambda c: (loads[c], c))
        cores[c].append(int(n))
        loads[c] += deg[n]
    prof = np.zeros(SLOTS, np.int64)
    for c in range(NCORES):
        ds = np.array([deg[n] for n in cores[c]])
        prof = np.maximum(prof, ds)
    prof = prof.astype(int)
    e_pad = max(int(prof.sum()), 1)
    starts = np.concatenate([[0], np.cumsum(prof)]).astype(int)
    by_dst = [[] for _ in range(Bn)]
    for e in range(src.shape[0]):
        by_dst[int(dst[e])].append(int(src[e]))
    sched_src = np.zeros((NCORES, e_pad), np.int64)
    is_pad = np.ones((NCORES, e_pad), bool)
    degs = np.zeros((NCORES, SLOTS), np.int64)
    mask = np.zeros((NCORES, SLOTS), np.float32)
    for c in range(NCORES):
        for s in range(SLOTS):
            node = cores[c][s]
            lst = by_dst[node]
            degs[c, s] = len(lst)
            mask[c, s] = 1.0 if lst else 0.0
            for j in range(prof[s]):
                p = starts[s] + j
                if j < len(lst):
                    sched_src[c, p] = lst[j]
                    is_pad[c, p] = False
    return cores, prof, e_pad, starts, sched_src, is_pad, degs, mask


def _build_kernel1():
    nc = bacc.Bacc("TRN2", target_bir_lowering=False, debug=False,
                   num_devices=NCORES)
    x_in = nc.dram_tensor("x_own", [TOK, C], F32, kind="ExternalInput")
    wqk = nc.dram_tensor("wqkT_aug", [C + 1, 2 * C], BF16, kind="ExternalInput")
    wv = nc.dram_tensor("wvT_aug", [C + 1, H * 65], BF16, kind="ExternalInput")
    ones_in = nc.dram_tensor("ones_row", [1, TOK], BF16, kind="ExternalInput")
    qt_out = nc.dram_tensor("qT_tab", [HD, H * TOK], BF16, kind="ExternalOutput")
    kt_out = nc.dram_tensor("kT_tab", [HD, H * TOK], BF16, kind="ExternalOutput")
    v_hi_out = nc.dram_tensor("v_hi", [128, SLOTS * 195], BF16, kind="ExternalOutput")
    v_lo_out = nc.dram_tensor("v_lo", [69, SLOTS * 195], BF16, kind="ExternalOutput")

    NT = (TOK + 127) // 128      # 50 token tiles (49 full + 32)

    with tile.TileContext(nc) as tc:
        with tc.tile_pool(name="cst", bufs=1) as cst:
            ident = cst.tile([128, 128], BF16)
            make_identity(nc, ident[:])
            eps_t = cst.tile([128, 1], F32)
            nc.vector.memset(eps_t[:], EPS)
            wqk_a = cst.tile([128, 2 * C], BF16)
            wqk_b = cst.tile([65, 2 * C], BF16)
            nc.sync.dma_start(wqk_a[:], wqk[0:128, :])
            nc.sync.dma_start(wqk_b[:], wqk[128:193, :])
            wv_a = cst.tile([128, H * 65], BF16)
            wv_b = cst.tile([65, H * 65], BF16)
            nc.sync.dma_start(wv_a[:], wv[0:128, :])
            nc.sync.dma_start(wv_b[:], wv[128:193, :])
            x_res = cst.tile([128, NT * C], F32)
            stats = cst.tile([128, NT, 2], F32)
            sd = cst.tile([128, NT, 1], F32)
            istd = cst.tile([128, NT, 1], F32)
            xhT_a = cst.tile([128, TOK], BF16)
            xhT_b = cst.tile([65, TOK], BF16)
            nc.sync.dma_start(xhT_b[64:65, :], ones_in[:])

            TGRP = 4
            with tc.tile_pool(name="pa", bufs=3) as sba, \
                 tc.tile_pool(name="pbt", bufs=1, space="PSUM") as pbt, \
                 tc.tile_pool(name="pc", bufs=3) as sbc, \
                 tc.tile_pool(name="pcp", bufs=2, space="PSUM") as pcp, \
                 tc.tile_pool(name="pd", bufs=3) as sbd, \
                 tc.tile_pool(name="pdp", bufs=2, space="PSUM") as pdp:
                # ---- pass A: load x (batched), LN stats ----
                for g0 in range(0, TOK, 512):
                    gl = min(512, TOK - g0)
                    if gl == 512:
                        nc.sync.dma_start(
                            x_res[:, g0 // 128 * C:(g0 // 128 + 4) * C]
                            .rearrange("p (i c) -> p i c", c=C),
                            x_in[g0:g0 + 512, :].rearrange("(i p) c -> p i c", p=128))
                    else:
                        for t0 in range(g0, TOK, 128):
                            tl = min(128, TOK - t0)
                            nc.sync.dma_start(
                                x_res[0:tl, t0 // 128 * C:(t0 // 128 + 1) * C],
                                x_in[t0:t0 + tl, :])
                for t in range(NT):
                    tl = min(128, TOK - t * 128)
                    st6 = sba.tile([128, 6], F32, tag="st6")
                    nc.vector.bn_stats(st6[0:tl, :], x_res[0:tl, t * C:(t + 1) * C])
                    nc.vector.bn_aggr(stats[0:tl, t, :], st6[0:tl, :])
                nc.scalar.activation(sd[:], stats[:, :, 1:2],
                                     mybir.ActivationFunctionType.Sqrt,
                                     bias=eps_t[:])
                nc.vector.reciprocal(istd[:], sd[:])

                # ---- pass B: xhat + transpose ----
                for t in range(NT):
                    g0 = t * 128
                    tl = min(128, TOK - g0)
                    xh = sba.tile([128, C], BF16, tag="xh")
                    nc.vector.tensor_scalar(xh[0:tl, :], x_res[0:tl, t * C:(t + 1) * C],
                                            stats[0:tl, t, 0:1], istd[0:tl, t, :],
                                            mybir.AluOpType.subtract,
                                            mybir.AluOpType.mult)
                    tp0 = pbt.tile([128, 128], BF16, tag="tp0")
                    tp1 = pbt.tile([64, 128], BF16, tag="tp1")
                    nc.tensor.transpose(tp0[:, 0:tl], xh[0:tl, 0:128], ident[0:tl, 0:tl])
                    nc.tensor.transpose(tp1[:, 0:tl], xh[0:tl, 128:192], ident[0:tl, 0:tl])
                    nc.vector.tensor_copy(out=xhT_a[:, g0:g0 + tl], in_=tp0[:, 0:tl])
                    nc.scalar.copy(out=xhT_b[0:64, g0:g0 + tl], in_=tp1[:, 0:tl])

                # ---- pass C: q/k projections (h-major tables) ----
                for gg in range(0, NT, TGRP):
                    gn = min(TGRP, NT - gg)
                    for cc in range(3):
                        qkp = pcp.tile([128, TGRP * 128], F32, tag="qkp",
                                       name=f"qkp_{gg}_{cc}")
                        for tt in range(gn):
                            g0 = (gg + tt) * 128
                            tl = min(128, TOK - g0)
                            nc.tensor.matmul(qkp[:, tt * 128:tt * 128 + tl],
                                             wqk_a[:, cc * 128:(cc + 1) * 128],
                                             xhT_a[:, g0:g0 + tl],
                                             start=True, stop=False)
                            nc.tensor.matmul(qkp[:, tt * 128:tt * 128 + tl],
                                             wqk_b[:, cc * 128:(cc + 1) * 128],
                                             xhT_b[:, g0:g0 + tl],
                                             start=False, stop=True)
                        g0 = gg * 128
                        glen = min(TGRP * 128, TOK - g0)
                        for half in range(2):
                            gidx = cc * 2 + half
                            if gidx < 3:
                                dstt, hh = qt_out, gidx
                            else:
                                dstt, hh = kt_out, gidx - 3
                            stg = sbc.tile([64, TGRP * 128], BF16, tag="stg")
                            if half == 0:
                                nc.vector.tensor_copy(out=stg[:, 0:glen],
                                                      in_=qkp[0:64, 0:glen])
                            else:
                                nc.scalar.copy(out=stg[:, 0:glen],
                                               in_=qkp[64:128, 0:glen])
                            nc.sync.dma_start(
                                dstt[:, hh * TOK + g0: hh * TOK + g0 + glen],
                                stg[:, 0:glen])

                # ---- pass D: v projection (token-major per slot) ----
                for s in range(SLOTS):
                    for mi, (m0, ml) in enumerate(MC):
                        r0 = s * N + m0
                        vp = pdp.tile([128, H * 65], F32, tag="vp")
                        nc.tensor.matmul(vp[0:ml, :], xhT_a[:, r0:r0 + ml], wv_a[:],
                                         start=True, stop=False)
                        nc.tensor.matmul(vp[0:ml, :], xhT_b[:, r0:r0 + ml], wv_b[:],
                                         start=False, stop=True)
                        vsb = sbd.tile([128, H * 65], BF16, tag="vsb")
                        if mi == 0:
                            nc.vector.tensor_copy(out=vsb[0:ml, :], in_=vp[0:ml, :])
                        else:
                            nc.scalar.copy(out=vsb[0:ml, :], in_=vp[0:ml, :])
                        dstt = v_hi_out if mi == 0 else v_lo_out
                        nc.sync.dma_start(dstt[0:ml, s * 195:(s + 1) * 195],
                                          vsb[0:ml, :])
    nc.compile()
    return nc


def _build_kernel2(prof, e_pad):
    starts = np.concatenate([[0], np.cumsum(prof)]).astype(int)
    nc = bacc.Bacc("TRN2", target_bir_lowering=False, debug=False,
                   num_devices=NCORES)
    x_in = nc.dram_tensor("x_own", [TOK + 64, C], F32, kind="ExternalInput")
    qt_in = nc.dram_tensor("qT_aug", [HD + 1, H * TOK], BF16, kind="ExternalInput")
    v_hi_in = nc.dram_tensor("v_hi_aug", [128, SLOTS * 195], BF16, kind="ExternalInput")
    v_lo_in = nc.dram_tensor("v_lo_aug", [70, SLOTS * 195], BF16, kind="ExternalInput")
    kte_in = nc.dram_tensor("kT_edges", [e_pad * (HD + 1), KCOLS], BF16,
                            kind="ExternalInput")
    mrow_in = nc.dram_tensor("maskrow", [1, TOK], BF16, kind="ExternalInput")
    ones_in = nc.dram_tensor("ones_row", [1, TOK], BF16, kind="ExternalInput")
    pw_in = nc.dram_tensor("projWT", [C, C], BF16, kind="ExternalInput")
    pb_in = nc.dram_tensor("projb", [1, C], BF16, kind="ExternalInput")
    w1_in = nc.dram_tensor("w1T_aug", [C + 1, HID], BF16, kind="ExternalInput")
    w2_in = nc.dram_tensor("w2T_aug", [HID + 1, C], BF16, kind="ExternalInput")
    out = nc.dram_tensor("out_own", [TOK, C], F32, kind="ExternalOutput")

    NHC = [(0, 128), (128, 69)]   # n-chunks (dest tokens) within a node
    NT = (TOK + 127) // 128

    with tile.TileContext(nc) as tc:
        with tc.tile_pool(name="cst", bufs=1) as cst, \
             tc.tile_pool(name="dram", bufs=1, space="DRAM") as dpool:
            x2_dram = dpool.tile([TOK + 96, C], F32)
            ident_bf = cst.tile([128, 128], BF16)
            make_identity(nc, ident_bf[:])
            eps_t = cst.tile([128, 1], F32)
            nc.vector.memset(eps_t[:], EPS)
            qt_sb = cst.tile([HD + 1, H * TOK], BF16)
            nc.sync.dma_start(qt_sb[:], qt_in[:])
            v_hi = cst.tile([128, SLOTS * 195], BF16)
            v_lo = cst.tile([70, SLOTS * 195], BF16)
            nc.sync.dma_start(v_hi[:], v_hi_in[:])
            nc.sync.dma_start(v_lo[:], v_lo_in[:])
            mrow = cst.tile([1, TOK], BF16)
            nc.sync.dma_start(mrow[:], mrow_in[:])
            pw_a = cst.tile([128, C], BF16)
            pw_b = cst.tile([64, C], BF16)
            nc.sync.dma_start(pw_a[:], pw_in[0:128, :])
            nc.sync.dma_start(pw_b[:], pw_in[128:192, :])
            pb_sb = cst.tile([1, C], BF16)
            nc.sync.dma_start(pb_sb[:], pb_in[:])
            acc = cst.tile([128, SLOTS * 2 * C], F32)
            nc.vector.memset(acc[:], 0.0)
            stats = cst.tile([128, 2 * SLOTS, 2], F32)
            sd = cst.tile([128, 2 * SLOTS, 1], F32)
            istd = cst.tile([128, 2 * SLOTS, 1], F32)
            xh2T_a = cst.tile([128, TOK], BF16)
            xh2T_b = cst.tile([65, TOK], BF16)
            nc.sync.dma_start(xh2T_b[64:65, :], ones_in[:])

            # ---------------- phase B: per-edge attention ----------------
            edges = [(s, starts[s] + j) for s in range(SLOTS)
                     for j in range(int(prof[s]))]
            with tc.tile_pool(name="pb_sb", bufs=3) as sbb, \
                 tc.tile_pool(name="ps_s", bufs=2, space="PSUM") as ps_s, \
                 tc.tile_pool(name="ps_m", bufs=2, space="PSUM") as ps_m:

                def emit_qk_exp(s, ep):
                    kst = sbb.tile([HD + 1, KCOLS], BF16, tag="kst",
                                   name=f"kst_{ep}")
                    nc.sync.dma_start(kst[:], kte_in[ep * 65:(ep + 1) * 65, :])
                    S = ps_s.tile([128, 3 * 512], F32, tag="S", name=f"S_{ep}")
                    for h in range(H):
                        nc.tensor.matmul(
                            S[0:128, h * 512: h * 512 + N],
                            kst[:, h * KC: h * KC + 128],
                            qt_sb[:, h * TOK + s * N: h * TOK + (s + 1) * N],
                            start=True, stop=True)
                        nc.tensor.matmul(
                            S[0:70, h * 512 + N: h * 512 + 2 * N],
                            kst[:, h * KC + 128: h * KC + KC],
                            qt_sb[:, h * TOK + s * N: h * TOK + (s + 1) * N],
                            start=True, stop=True)
                    E = sbb.tile([128, 3, 2 * N], BF16, tag="E", name=f"E_{ep}")
                    nc.scalar.activation(
                        E[:],
                        S[:].rearrange("p (h c) -> p h c", h=3)[:, :, 0:2 * N],
                        mybir.ActivationFunctionType.Exp, scale=SCALE)
                    return S, E

                def emit_av_norm(s, ep, E):
                    msg = ps_m.tile([128, 2 * 195], F32, tag="msg",
                                    name=f"msg_{ep}")
                    for ni, (n0, nl) in enumerate(NHC):
                        for h in range(H):
                            nc.tensor.matmul(
                                msg[0:nl, ni * 195 + h * 65: ni * 195 + (h + 1) * 65],
                                E[0:128, h, n0: n0 + nl],
                                v_hi[0:128, s * 195 + h * 65: s * 195 + (h + 1) * 65],
                                start=True, stop=False)
                            nc.tensor.matmul(
                                msg[0:nl, ni * 195 + h * 65: ni * 195 + (h + 1) * 65],
                                E[0:70, h, N + n0: N + n0 + nl],
                                v_lo[0:70, s * 195 + h * 65: s * 195 + (h + 1) * 65],
                                start=False, stop=True)
                    rec = sbb.tile([128, 2, 3, 1], F32, tag="rec", name=f"rec_{ep}")
                    nc.vector.reciprocal(
                        rec[:],
                        msg[:].rearrange("p (i h c) -> p i h c", i=2, c=65)[:, :, :, 64:65])
                    for ni, (n0, nl) in enumerate(NHC):
                        for h in range(H):
                            a_sl = acc[0:nl,
                                       s * 2 * C + ni * C + h * 64:
                                       s * 2 * C + ni * C + (h + 1) * 64]
                            nc.vector.scalar_tensor_tensor(
                                out=a_sl,
                                in0=msg[0:nl, ni * 195 + h * 65: ni * 195 + h * 65 + 64],
                                scalar=rec[0:nl, ni, h, :],
                                in1=a_sl,
                                op0=mybir.AluOpType.mult,
                                op1=mybir.AluOpType.add)

                prev = None
                for (s, ep) in edges:
                    S, E = emit_qk_exp(s, ep)
                    if prev is not None:
                        emit_av_norm(*prev)
                    prev = (s, ep, E)
                emit_av_norm(*prev)

            # ---------------- phase C: proj + residual + LN2 stats ----------------
            with tc.tile_pool(name="pc_sb", bufs=3) as sbc, \
                 tc.tile_pool(name="ps_t", bufs=2, space="PSUM") as ps_t, \
                 tc.tile_pool(name="ps_c", bufs=2, space="PSUM") as ps_c:
                for s in range(SLOTS):
                    acc_bf = sbc.tile([128, 2 * C], BF16, tag="accbf")
                    nc.vector.tensor_copy(out=acc_bf[:],
                                          in_=acc[:, s * 2 * C:(s + 1) * 2 * C])
                    xt2 = sbc.tile([128, 2, C], F32, tag="xt2")
                    nc.sync.dma_start(
                        xt2[:],
                        x_in[s * N: s * N + 256, :]
                        .rearrange("(i p) c -> p i c", p=128))
                    agT_a = sbc.tile([128, N], BF16, tag="agTa")
                    agT_b = sbc.tile([64, N], BF16, tag="agTb")
                    for ni, (n0, nl) in enumerate(NHC):
                        tp0 = ps_t.tile([128, 128], BF16, tag="tp0")
                        tp1 = ps_t.tile([64, 128], BF16, tag="tp1")
                        nc.tensor.transpose(tp0[:, 0:nl], acc_bf[0:nl, ni * C:ni * C + 128],
                                            ident_bf[0:nl, 0:nl])
                        nc.tensor.transpose(tp1[:, 0:nl], acc_bf[0:nl, ni * C + 128:ni * C + 192],
                                            ident_bf[0:nl, 0:nl])
                        nc.vector.tensor_copy(out=agT_a[:, n0:n0 + nl], in_=tp0[:, 0:nl])
                        nc.scalar.copy(out=agT_b[:, n0:n0 + nl], in_=tp1[:, 0:nl])
                    for ni, (n0, nl) in enumerate(NHC):
                        yp = ps_c.tile([128, C], F32, tag="yp")
                        nc.tensor.matmul(yp[0:nl, :], agT_a[:, n0:n0 + nl], pw_a[:],
                                         start=True, stop=False)
                        nc.tensor.matmul(yp[0:nl, :], agT_b[:, n0:n0 + nl], pw_b[:],
                                         start=False, stop=False)
                        nc.tensor.matmul(yp[0:nl, :],
                                         mrow[0:1, s * N + n0: s * N + n0 + nl],
                                         pb_sb[:], start=False, stop=True)
                        r0 = s * N + n0
                        x2t = sbc.tile([128, C], F32, tag="x2t")
                        nc.vector.tensor_tensor(out=x2t[0:nl, :], in0=yp[0:nl, :],
                                                in1=xt2[0:nl, ni, :],
                                                op=mybir.AluOpType.add)
                        nc.sync.dma_start(x2_dram[r0:r0 + nl, :], x2t[0:nl, :])
                        st6 = sbc.tile([128, 6], F32, tag="st6")
                        nc.vector.bn_stats(st6[0:nl, :], x2t[0:nl, :])
                        nc.vector.bn_aggr(stats[0:nl, s * 2 + ni, :], st6[0:nl, :])

            nc.scalar.activation(sd[:], stats[:, :, 1:2],
                                 mybir.ActivationFunctionType.Sqrt,
                                 bias=eps_t[:])
            nc.vector.reciprocal(istd[:], sd[:])

            # ---------------- phase C2: xh2T build ----------------
            with tc.tile_pool(name="c2_sb", bufs=3) as sb2, \
                 tc.tile_pool(name="c2_ps", bufs=2, space="PSUM") as ps2:
                for s in range(SLOTS):
                    for ni, (n0, nl) in enumerate(NHC):
                        r0 = s * N + n0
                        xt2 = sb2.tile([128, C], F32, tag="xt2")
                        nc.sync.dma_start(xt2[0:nl, :], x2_dram[r0:r0 + nl, :])
                        xh2 = sb2.tile([128, C], BF16, tag="xh2")
                        nc.vector.tensor_scalar(xh2[0:nl, :], xt2[0:nl, :],
                                                stats[0:nl, s * 2 + ni, 0:1],
                                                istd[0:nl, s * 2 + ni, :],
                                                mybir.AluOpType.subtract,
                                                mybir.AluOpType.mult)
                        tp0 = ps2.tile([128, 128], BF16, tag="tp0")
                        tp1 = ps2.tile([64, 128], BF16, tag="tp1")
                        nc.tensor.transpose(tp0[:, 0:nl], xh2[0:nl, 0:128],
                                            ident_bf[0:nl, 0:nl])
                        nc.tensor.transpose(tp1[:, 0:nl], xh2[0:nl, 128:192],
                                            ident_bf[0:nl, 0:nl])
                        nc.vector.tensor_copy(out=xh2T_a[:, r0:r0 + nl], in_=tp0[:, 0:nl])
                        nc.scalar.copy(out=xh2T_b[0:64, r0:r0 + nl], in_=tp1[:, 0:nl])

            # ---------------- phase D: MLP ----------------
            with tc.tile_pool(name="pd_cst", bufs=1) as cd, \
                 tc.tile_pool(name="pd_sb", bufs=3) as sbd, \
                 tc.tile_pool(name="ps_d", bufs=2, space="PSUM") as ps_d:
                w1_a = cd.tile([128, HID], BF16)
                w1_b = cd.tile([65, HID], BF16)
                nc.sync.dma_start(w1_a[:], w1_in[0:128, :])
                nc.sync.dma_start(w1_b[:], w1_in[128:193, :])
                w2_t = []
                for hc in range(6):
                    t = cd.tile([128, C], BF16, tag=f"w2_{hc}", name=f"w2_{hc}")
                    nc.sync.dma_start(t[:], w2_in[hc * 128:(hc + 1) * 128, :])
                    w2_t.append(t)
                w2_bias = cd.tile([1, C], BF16)
                nc.sync.dma_start(w2_bias[:], w2_in[HID:HID + 1, :])
                ones_sb = cd.tile([1, 128], BF16)
                nc.vector.memset(ones_sb[:], 1.0)

                BLK = 512
                for b0 in range(0, TOK, BLK):
                    bl = min(BLK, TOK - b0)
                    h1 = [sbd.tile([128, BLK], BF16, tag=f"h1_{hc}", name=f"h1_{hc}_{b0}")
                          for hc in range(6)]
                    for hc in range(6):
                        hp = ps_d.tile([128, BLK], F32, tag="hp", name=f"hp_{hc}_{b0}")
                        nc.tensor.matmul(hp[:, 0:bl], w1_a[:, hc * 128:(hc + 1) * 128],
                                         xh2T_a[:, b0:b0 + bl], start=True, stop=False)
                        nc.tensor.matmul(hp[:, 0:bl], w1_b[:, hc * 128:(hc + 1) * 128],
                                         xh2T_b[:, b0:b0 + bl], start=False, stop=True)
                        nc.scalar.activation(h1[hc][:, 0:bl], hp[:, 0:bl],
                                             mybir.ActivationFunctionType.Gelu)
                    for u0 in range(0, bl, 256):
                        gu = b0 + u0
                        x2t = sbd.tile([128, 2, C], F32, tag="x2t",
                                       name=f"x2t_{gu}")
                        nc.sync.dma_start(
                            x2t[:],
                            x2_dram[gu:gu + 256, :]
                            .rearrange("(i p) c -> p i c", p=128))
                        ot = sbd.tile([128, 2, C], F32, tag="ot", name=f"ot_{gu}")
                        for v in range(2):
                            t0 = u0 + v * 128
                            if t0 >= bl:
                                continue
                            tl = min(128, bl - t0)
                            op = ps_d.tile([128, C], F32, tag="op",
                                           name=f"op_{b0}_{t0}")
                            for hc in range(6):
                                nc.tensor.matmul(op[0:tl, :], h1[hc][:, t0:t0 + tl],
                                                 w2_t[hc][:], start=(hc == 0),
                                                 stop=False)
                            nc.tensor.matmul(op[0:tl, :],
                                             ones_sb[0:1, 0:tl],
                                             w2_bias[:], start=False, stop=True)
                            nc.vector.tensor_tensor(out=ot[0:tl, v, :],
                                                    in0=op[0:tl, :],
                                                    in1=x2t[0:tl, v, :],
                                                    op=mybir.AluOpType.add)
                        rem = min(256, TOK - gu)
                        if rem == 256:
                            nc.sync.dma_start(
                                out[gu:gu + 256, :]
                                .rearrange("(i p) c -> p i c", p=128),
                                ot[:])
                        else:
                            nc.sync.dma_start(out[gu:gu + 128, :], ot[:, 0, :])
                            if rem > 128:
                                nc.sync.dma_start(out[gu + 128:gu + rem, :],
                                                  ot[0:rem - 128, 1, :])
    nc.compile()
    return nc


def kernel(x, egde, norm1_g, norm1_b, qkv_w, proj_w, proj_b,
           norm2_g, norm2_b, fc1_w, fc1_b, fc2_w, fc2_b):
    x = np.asarray(x, np.float32)
    edge = np.asarray(egde)
    g1 = np.asarray(norm1_g, np.float32)
    b1 = np.asarray(norm1_b, np.float32)
    qkv_w = np.asarray(qkv_w, np.float32)

    cores, prof, e_pad, starts, sched_src, is_pad, degs, mask = _plan(edge)

    # ---- kernel 1 host prep ----
    wqk = (qkv_w[0:2 * C, :] * g1[None, :]).T             # [C, 384]
    bqk = qkv_w[0:2 * C, :] @ b1
    wqkT_aug = _bf(np.concatenate([wqk, bqk[None, :]], 0))
    # v weights rearranged to (h, 65) with zero rowsum columns
    wv = (qkv_w[2 * C:3 * C, :] * g1[None, :]).T          # [C, 192]
    bv = qkv_w[2 * C:3 * C, :] @ b1                       # [192]
    wv_aug = np.zeros((C + 1, H * 65), np.float32)
    for h in range(H):
        wv_aug[0:C, h * 65:h * 65 + 64] = wv[:, h * 64:(h + 1) * 64]
        wv_aug[C, h * 65:h * 65 + 64] = bv[h * 64:(h + 1) * 64]
    wvT_aug = _bf(wv_aug)
    ones_row = _bf(np.ones((1, TOK), np.float32))

    x_own = np.stack([x[cores[c]].reshape(TOK, C) for c in range(NCORES)])

    nc1 = _build_kernel1()
    in_maps1 = [{"x_own": np.ascontiguousarray(x_own[c]),
                 "wqkT_aug": wqkT_aug, "wvT_aug": wvT_aug,
                 "ones_row": ones_row}
                for c in range(NCORES)]
    res1 = bass_utils.run_bass_kernel_spmd(nc1, in_maps1, core_ids=list(range(NCORES)),
                                           trace=TRACE)

    # ---- host gather: build per-edge k tables + augmented inputs ----
    # kT_tab: [64, (h, TOK)] -> global [64, H, Bn, N]
    kt_glob = np.zeros((HD, H, Bn, N), bf)
    for c in range(NCORES):
        sh = res1.results[c]["kT_tab"].reshape(HD, H, SLOTS, N)
        for s in range(SLOTS):
            kt_glob[:, :, cores[c][s], :] = sh[:, :, s, :]
    kte = np.zeros((NCORES, e_pad * (HD + 1), KCOLS), bf)
    beta_bf = np.array(BETA, bf)
    for c in range(NCORES):
        gathered = kt_glob[:, :, sched_src[c], :]         # [64, H, e_pad, N]
        blk = np.zeros((e_pad, HD + 1, H, KC), bf)
        blk[:, 0:HD, :, 0:N] = gathered.transpose(2, 0, 1, 3)
        # beta row: -1e5 on real m-cols of pad edges; 0 on fake/pad cols
        blk[is_pad[c], HD, :, 0:N] = beta_bf
        kte[c] = blk.reshape(e_pad * (HD + 1), KCOLS)

    # q tables + ones contract row
    qt_aug = np.zeros((NCORES, HD + 1, H * TOK), bf)
    for c in range(NCORES):
        qt_aug[c, 0:HD] = res1.results[c]["qT_tab"]
        qt_aug[c, HD] = np.array(1.0, bf)

    # v tables: deg-scaled rowsum columns + fake row (eps)
    v_hi_aug = np.zeros((NCORES, 128, SLOTS * 195), bf)
    v_lo_aug = np.zeros((NCORES, 70, SLOTS * 195), bf)
    fake = np.zeros((SLOTS * 195,), np.float32)
    for c in range(NCORES):
        vh = np.asarray(res1.results[c]["v_hi"], bf).copy()
        vl = np.asarray(res1.results[c]["v_lo"], bf).copy()
        for s in range(SLOTS):
            d = float(degs[c, s])
            for h in range(H):
                col = s * 195 + h * 65 + 64
                vh[:, col] = np.array(d, bf)
                vl[:, col] = np.array(d, bf)
                fake[col] = FAKE_EPS
        v_hi_aug[c, :, :] = vh
        v_lo_aug[c, 0:69, :] = vl
        v_lo_aug[c, 69, :] = fake.astype(bf)

    # ---- kernel 2 host prep ----
    g2 = np.asarray(norm2_g, np.float32)
    b2 = np.asarray(norm2_b, np.float32)
    fc1_w = np.asarray(fc1_w, np.float32)
    fc2_w = np.asarray(fc2_w, np.float32)
    w1 = (fc1_w * g2[None, :]).T
    bb1 = fc1_w @ b2 + np.asarray(fc1_b, np.float32)
    w1T_aug = _bf(np.concatenate([w1, bb1[None, :]], 0))
    w2T_aug = _bf(np.concatenate([fc2_w.T, np.asarray(fc2_b, np.float32)[None, :]], 0))
    projWT = _bf(np.asarray(proj_w, np.float32).T)
    projb = _bf(np.asarray(proj_b, np.float32)[None, :])
    maskrow = _bf(np.repeat(mask, N, axis=1)[:, None, :])

    x_own_pad = np.zeros((NCORES, TOK + 64, C), np.float32)
    x_own_pad[:, 0:TOK, :] = x_own

    nc2 = _build_kernel2(prof, e_pad)
    in_maps2 = []
    for c in range(NCORES):
        in_maps2.append({
            "x_own": np.ascontiguousarray(x_own_pad[c]),
            "qT_aug": np.ascontiguousarray(qt_aug[c]),
            "v_hi_aug": np.ascontiguousarray(v_hi_aug[c]),
            "v_lo_aug": np.ascontiguousarray(v_lo_aug[c]),
            "kT_edges": np.ascontiguousarray(kte[c]),
            "maskrow": np.ascontiguousarray(maskrow[c]),
            "ones_row": ones_row,
            "projWT": projWT, "projb": projb,
            "w1T_aug": w1T_aug, "w2T_aug": w2T_aug,
        })
    res2 = bass_utils.run_bass_kernel_spmd(nc2, in_maps2, core_ids=list(range(NCORES)),
                                           trace=TRACE)
    global LAST_EXEC_NS
    LAST_EXEC_NS = [res1.exec_time_ns or 0, res2.exec_time_ns or 0]

    outp = np.zeros((Bn, N, C), np.float32)
    for c in range(NCORES):
        outp[cores[c]] = res2.results[c]["out_own"].reshape(SLOTS, N, C)
    return outp


# revision 22
# speedup vs baseline: 1.2088x; 1.0474x over previous
"""Trainium2 Bass kernel for nn_Block_21792664060277 (gnn_message_passing).

Strategy (8 NeuronCores, SPMD):
  - Destination-node sharding: 256 graph nodes -> 8 cores x 32 slots,
    greedy-balanced by in-degree; scatter-mean becomes core-local.
  - Kernel 1 (per core, own 32 nodes): LayerNorm (stats via bn_stats,
    weights folded), qkv projection producing h-major transposed q/k
    tables [64, (h, tok)] and token-major v tables.
  - Host: gathers k tables into per-edge order, appends an augmented
    contract row (beta row: 0 for real edges, -1e5 for padding edges so
    exp underflows to exactly 0) and a fake m-column per head whose V
    row is (0..0, eps) so padded rowsums are eps (no NaN); scales the
    V ones-column by deg so the reciprocal of the rowsum directly gives
    the scatter-mean weight.
  - Kernel 2 (per core): per-edge QK^T (contract 65) -> ONE exp
    activation per edge over all heads/chunks -> AV with rowsum column
    -> reciprocal + 6 scalar_tensor_tensor accumulates; then proj
    (+masked bias), residual, LN2 + MLP (gelu), final residual.
"""
import sys

if "/opt/trn_rl_repo" not in sys.path:
    sys.path.insert(0, "/opt/trn_rl_repo")


def _ensure_ntff_hook():
    """Register the axon NTFF profiling hook if the antenv.axon_hooks
    shim module is absent (dropped from some images); without it
    run_bass_kernel_spmd(trace=True) silently skips profiling."""
    try:
        import antenv.axon_hooks  # noqa: F401
        return
    except ImportError:
        pass
    try:
        import types
        import antenv
        mod = types.ModuleType("antenv.axon_hooks")
        _h = {"hook": None}

        def set_axon_ntff_profile_hook(hook):
            _h["hook"] = hook

        def get_axon_ntff_profile_hook():
            return _h["hook"]

        mod.set_axon_ntff_profile_hook = set_axon_ntff_profile_hook
        mod.get_axon_ntff_profile_hook = get_axon_ntff_profile_hook
        sys.modules["antenv.axon_hooks"] = mod
        antenv.axon_hooks = mod
        from trn_agent_boot.trn_boot import _ntff_profile_via_ctypes
        hook = _ntff_profile_via_ctypes("/opt/axon/libaxon_pjrt.so")
        if hook is not None:
            set_axon_ntff_profile_hook(hook)
    except Exception:
        pass


_ensure_ntff_hook()

import numpy as np
import ml_dtypes

import concourse.bass as bass  # noqa: F401
import concourse.bacc as bacc
import concourse.mybir as mybir
import concourse.tile as tile
from concourse import bass_utils
from concourse.masks import make_identity

BF16 = mybir.dt.bfloat16
F32 = mybir.dt.float32

Bn, N, C = 256, 197, 192
H, HD = 3, 64
HID = 768
NCORES = 8
SLOTS = 32
TOK = SLOTS * N          # 6304
EPS = 1e-5
SCALE = HD ** -0.5
BETA = -1.0e5            # pad-edge logit offset (exp underflows to 0)
FAKE_EPS = 1e-30         # fake-column rowsum epsilon
KC = 198                 # per-head k-table cols (197 real + 1 fake)
KCOLS = H * KC           # 594: per-edge k table cols (h-major)

MC = [(0, 128), (128, 69)]      # token chunks within a node (197 = 128+69)

TRACE = False
LAST_EXEC_NS = None

bf = ml_dtypes.bfloat16


def _bf(a):
    return np.ascontiguousarray(np.asarray(a, np.float32)).astype(bf)


def _plan(edge):
    """Node->core assignment balanced by degree + shared degree profile."""
    src, dst = np.asarray(edge[0]), np.asarray(edge[1])
    deg = np.bincount(dst, minlength=Bn)
    order = np.argsort(-deg, kind="stable")
    cores = [[] for _ in range(NCORES)]
    loads = np.zeros(NCORES, np.int64)
    for n in order:
        free = [c for c in range(NCORES) if len(cores[c]) < SLOTS]
        c = min(free, key=lambda c: (loads[c], c))
        cores[c].append(int(n))
        loads[c] += deg[n]
    prof = np.zeros(SLOTS, np.int64)
    for c in range(NCORES):
        ds = np.array([deg[n] for n in cores[c]])
        prof = np.maximum(prof, ds)
    prof = prof.astype(int)
    e_pad = max(int(prof.sum()), 1)
    starts = np.concatenate([[0], np.cumsum(prof)]).astype(int)
    by_dst = [[] for _ in range(Bn)]
    for e in range(src.shape[0]):
        by_dst[int(dst[e])].append(int(src[e]))
    sched_src = np.zeros((NCORES, e_pad), np.int64)
    is_pad = np.ones((NCORES, e_pad), bool)
    degs = np.zeros((NCORES, SLOTS), np.int64)
    mask = np.zeros((NCORES, SLOTS), np.float32)
    for c in range(NCORES):
        for s in range(SLOTS):
            node = cores[c][s]
            lst = by_dst[node]
            degs[c, s] = len(lst)
            mask[c, s] = 1.0 if lst else 0.0
            for j in range(prof[s]):
                p = starts[s] + j
                if j < len(lst):
                    sched_src[c, p] = lst[j]
                    is_pad[c, p] = False
    return cores, prof, e_pad, starts, sched_src, is_pad, degs, mask


def _build_kernel1():
    nc = bacc.Bacc("TRN2", target_bir_lowering=False, debug=False,
                   num_devices=NCORES)
    x_in = nc.dram_tensor("x_own", [TOK, C], F32, kind="ExternalInput")
    wqk = nc.dram_tensor("wqkT_aug", [C + 1, 2 * C], BF16, kind="ExternalInput")
    wv = nc.dram_tensor("wvT_aug", [C + 1, H * 65], BF16, kind="ExternalInput")
    ones_in = nc.dram_tensor("ones_row", [1, TOK], BF16, kind="ExternalInput")
    qt_out = nc.dram_tensor("qT_tab", [HD, H * TOK], BF16, kind="ExternalOutput")
    kt_out = nc.dram_tensor("kT_tab", [HD, H * TOK], BF16, kind="ExternalOutput")
    v_hi_out = nc.dram_tensor("v_hi", [128, SLOTS * 195], BF16, kind="ExternalOutput")
    v_lo_out = nc.dram_tensor("v_lo", [69, SLOTS * 195], BF16, kind="ExternalOutput")

    NT = (TOK + 127) // 128      # 50 token tiles (49 full + 32)

    with tile.TileContext(nc) as tc:
        with tc.tile_pool(name="cst", bufs=1) as cst:
            ident = cst.tile([128, 128], BF16)
            make_identity(nc, ident[:])
            eps_t = cst.tile([128, 1], F32)
            nc.vector.memset(eps_t[:], EPS)
            wqk_a = cst.tile([128, 2 * C], BF16)
            wqk_b = cst.tile([65, 2 * C], BF16)
            nc.sync.dma_start(wqk_a[:], wqk[0:128, :])
            nc.sync.dma_start(wqk_b[:], wqk[128:193, :])
            wv_a = cst.tile([128, H * 65], BF16)
            wv_b = cst.tile([65, H * 65], BF16)
            nc.sync.dma_start(wv_a[:], wv[0:128, :])
            nc.sync.dma_start(wv_b[:], wv[128:193, :])
            x_res = cst.tile([128, NT * C], F32)
            stats = cst.tile([128, NT, 2], F32)
            sd = cst.tile([128, NT, 1], F32)
            istd = cst.tile([128, NT, 1], F32)
            xhT_a = cst.tile([128, TOK], BF16)
            xhT_b = cst.tile([65, TOK], BF16)
            nc.sync.dma_start(xhT_b[64:65, :], ones_in[:])

            TGRP = 4
            with tc.tile_pool(name="pa", bufs=3) as sba, \
                 tc.tile_pool(name="pbt", bufs=1, space="PSUM") as pbt, \
                 tc.tile_pool(name="pc", bufs=3) as sbc, \
                 tc.tile_pool(name="pcp", bufs=2, space="PSUM") as pcp, \
                 tc.tile_pool(name="pd", bufs=3) as sbd, \
                 tc.tile_pool(name="pdp", bufs=2, space="PSUM") as pdp:
                # ---- pass A: load x (batched), LN stats ----
                for g0 in range(0, TOK, 512):
                    gl = min(512, TOK - g0)
                    if gl == 512:
                        nc.sync.dma_start(
                            x_res[:, g0 // 128 * C:(g0 // 128 + 4) * C]
                            .rearrange("p (i c) -> p i c", c=C),
                            x_in[g0:g0 + 512, :].rearrange("(i p) c -> p i c", p=128))
                    else:
                        for t0 in range(g0, TOK, 128):
                            tl = min(128, TOK - t0)
                            nc.sync.dma_start(
                                x_res[0:tl, t0 // 128 * C:(t0 // 128 + 1) * C],
                                x_in[t0:t0 + tl, :])
                for t in range(NT):
                    tl = min(128, TOK - t * 128)
                    st6 = sba.tile([128, 6], F32, tag="st6")
                    nc.vector.bn_stats(st6[0:tl, :], x_res[0:tl, t * C:(t + 1) * C])
                    nc.vector.bn_aggr(stats[0:tl, t, :], st6[0:tl, :])
                nc.scalar.activation(sd[:], stats[:, :, 1:2],
                                     mybir.ActivationFunctionType.Sqrt,
                                     bias=eps_t[:])
                nc.vector.reciprocal(istd[:], sd[:])

                # ---- pass B: xhat + transpose ----
                for t in range(NT):
                    g0 = t * 128
                    tl = min(128, TOK - g0)
                    xh = sba.tile([128, C], BF16, tag="xh")
                    nc.vector.tensor_scalar(xh[0:tl, :], x_res[0:tl, t * C:(t + 1) * C],
                                            stats[0:tl, t, 0:1], istd[0:tl, t, :],
                                            mybir.AluOpType.subtract,
                                            mybir.AluOpType.mult)
                    tp0 = pbt.tile([128, 128], BF16, tag="tp0")
                    tp1 = pbt.tile([64, 128], BF16, tag="tp1")
                    nc.tensor.transpose(tp0[:, 0:tl], xh[0:tl, 0:128], ident[0:tl, 0:tl])
                    nc.tensor.transpose(tp1[:, 0:tl], xh[0:tl, 128:192], ident[0:tl, 0:tl])
                    nc.vector.tensor_copy(out=xhT_a[:, g0:g0 + tl], in_=tp0[:, 0:tl])
                    nc.scalar.copy(out=xhT_b[0:64, g0:g0 + tl], in_=tp1[:, 0:tl])

                # ---- pass C: q/k projections (h-major tables) ----
                for gg in range(0, NT, TGRP):
                    gn = min(TGRP, NT - gg)
                    for cc in range(3):
                        qkp = pcp.tile([128, TGRP * 128], F32, tag="qkp",
                                       name=f"qkp_{gg}_{cc}")
                        for tt in range(gn):
                            g0 = (gg + tt) * 128
                            tl = min(128, TOK - g0)
                            nc.tensor.matmul(qkp[:, tt * 128:tt * 128 + tl],
                                             wqk_a[:, cc * 128:(cc + 1) * 128],
                                             xhT_a[:, g0:g0 + tl],
                                             start=True, stop=False)
                            nc.tensor.matmul(qkp[:, tt * 128:tt * 128 + tl],
                                             wqk_b[:, cc * 128:(cc + 1) * 128],
                                             xhT_b[:, g0:g0 + tl],
                                             start=False, stop=True)
                        g0 = gg * 128
                        glen = min(TGRP * 128, TOK - g0)
                        for half in range(2):
                            gidx = cc * 2 + half
                            if gidx < 3:
                                dstt, hh = qt_out, gidx
                            else:
                                dstt, hh = kt_out, gidx - 3
                            stg = sbc.tile([64, TGRP * 128], BF16, tag="stg")
                            if half == 0:
                                nc.vector.tensor_copy(out=stg[:, 0:glen],
                                                      in_=qkp[0:64, 0:glen])
                            else:
                                nc.scalar.copy(out=stg[:, 0:glen],
                                               in_=qkp[64:128, 0:glen])
                            nc.sync.dma_start(
                                dstt[:, hh * TOK + g0: hh * TOK + g0 + glen],
                                stg[:, 0:glen])

                # ---- pass D: v projection (token-major per slot) ----
                for s in range(SLOTS):
                    for mi, (m0, ml) in enumerate(MC):
                        r0 = s * N + m0
                        vp = pdp.tile([128, H * 65], F32, tag="vp")
                        nc.tensor.matmul(vp[0:ml, :], xhT_a[:, r0:r0 + ml], wv_a[:],
                                         start=True, stop=False)
                        nc.tensor.matmul(vp[0:ml, :], xhT_b[:, r0:r0 + ml], wv_b[:],
                                         start=False, stop=True)
                        vsb = sbd.tile([128, H * 65], BF16, tag="vsb")
                        if mi == 0:
                            nc.vector.tensor_copy(out=vsb[0:ml, :], in_=vp[0:ml, :])
                        else:
                            nc.scalar.copy(out=vsb[0:ml, :], in_=vp[0:ml, :])
                        dstt = v_hi_out if mi == 0 else v_lo_out
                        nc.sync.dma_start(dstt[0:ml, s * 195:(s + 1) * 195],
                                          vsb[0:ml, :])
    nc.compile()
    return nc


def _build_kernel2(prof, e_pad):
    starts = np.concatenate([[0], np.cumsum(prof)]).astype(int)
    nc = bacc.Bacc("TRN2", target_bir_lowering=False, debug=False,
                   num_devices=NCORES)
    x_in = nc.dram_tensor("x_own", [TOK + 64, C], F32, kind="ExternalInput")
    qt_in = nc.dram_tensor("qT_aug", [HD + 1, H * TOK], BF16, kind="ExternalInput")
    v_hi_in = nc.dram_tensor("v_hi_aug", [128, SLOTS * 195], BF16, kind="ExternalInput")
    v_lo_in = nc.dram_tensor("v_lo_aug", [70, SLOTS * 195], BF16, kind="ExternalInput")
    kte_in = nc.dram_tensor("kT_edges", [e_pad * (HD + 1), KCOLS], BF16,
                            kind="ExternalInput")
    mrow_in = nc.dram_tensor("maskrow", [1, TOK], BF16, kind="ExternalInput")
    ones_in = nc.dram_tensor("ones_row", [1, TOK], BF16, kind="ExternalInput")
    pw_in = nc.dram_tensor("projWT", [C, C], BF16, kind="ExternalInput")
    pb_in = nc.dram_tensor("projb", [1, C], BF16, kind="ExternalInput")
    w1_in = nc.dram_tensor("w1T_aug", [C + 1, HID], BF16, kind="ExternalInput")
    w2_in = nc.dram_tensor("w2T_aug", [HID + 1, C], BF16, kind="ExternalInput")
    out = nc.dram_tensor("out_own", [TOK, C], F32, kind="ExternalOutput")

    NHC = [(0, 128), (128, 69)]   # n-chunks (dest tokens) within a node
    NT = (TOK + 127) // 128

    with tile.TileContext(nc) as tc:
        with tc.tile_pool(name="cst", bufs=1) as cst, \
             tc.tile_pool(name="dram", bufs=1, space="DRAM") as dpool:
            x2_dram = dpool.tile([TOK + 96, C], F32)
            ident_bf = cst.tile([128, 128], BF16)
            make_identity(nc, ident_bf[:])
            eps_t = cst.tile([128, 1], F32)
            nc.vector.memset(eps_t[:], EPS)
            qt_sb = cst.tile([HD + 1, H * TOK], BF16)
            nc.sync.dma_start(qt_sb[:], qt_in[:])
            v_hi = cst.tile([128, SLOTS * 195], BF16)
            v_lo = cst.tile([70, SLOTS * 195], BF16)
            nc.sync.dma_start(v_hi[:], v_hi_in[:])
            nc.sync.dma_start(v_lo[:], v_lo_in[:])
            mrow = cst.tile([1, TOK], BF16)
            nc.sync.dma_start(mrow[:], mrow_in[:])
            pw_a = cst.tile([128, C], BF16)
            pw_b = cst.tile([64, C], BF16)
            nc.sync.dma_start(pw_a[:], pw_in[0:128, :])
            nc.sync.dma_start(pw_b[:], pw_in[128:192, :])
            pb_sb = cst.tile([1, C], BF16)
            nc.sync.dma_start(pb_sb[:], pb_in[:])
            acc = cst.tile([128, SLOTS * 2 * C], BF16)
            nc.vector.memset(acc[:], 0.0)
            x2_sb = cst.tile([128, SLOTS * 2 * C], BF16)
            stats = cst.tile([128, 2 * SLOTS, 2], F32)
            sd = cst.tile([128, 2 * SLOTS, 1], F32)
            istd = cst.tile([128, 2 * SLOTS, 1], F32)
            xh2T_a = cst.tile([128, TOK], BF16)
            xh2T_b = cst.tile([65, TOK], BF16)
            nc.sync.dma_start(xh2T_b[64:65, :], ones_in[:])

            # ---------------- phase B: per-edge attention ----------------
            edges = [(s, starts[s] + j) for s in range(SLOTS)
                     for j in range(int(prof[s]))]
            with tc.tile_pool(name="pb_sb", bufs=3) as sbb, \
                 tc.tile_pool(name="ps_s", bufs=2, space="PSUM") as ps_s, \
                 tc.tile_pool(name="ps_m", bufs=2, space="PSUM") as ps_m:

                def emit_qk_exp(s, ep):
                    kst = sbb.tile([HD + 1, KCOLS], BF16, tag="kst",
                                   name=f"kst_{ep}")
                    nc.sync.dma_start(kst[:], kte_in[ep * 65:(ep + 1) * 65, :])
                    S = ps_s.tile([128, 3 * 512], F32, tag="S", name=f"S_{ep}")
                    for h in range(H):
                        nc.tensor.matmul(
                            S[0:128, h * 512: h * 512 + N],
                            kst[:, h * KC: h * KC + 128],
                            qt_sb[:, h * TOK + s * N: h * TOK + (s + 1) * N],
                            start=True, stop=True)
                        nc.tensor.matmul(
                            S[0:70, h * 512 + N: h * 512 + 2 * N],
                            kst[:, h * KC + 128: h * KC + KC],
                            qt_sb[:, h * TOK + s * N: h * TOK + (s + 1) * N],
                            start=True, stop=True)
                    E = sbb.tile([128, 3, 2 * N], BF16, tag="E", name=f"E_{ep}")
                    nc.scalar.activation(
                        E[:],
                        S[:].rearrange("p (h c) -> p h c", h=3)[:, :, 0:2 * N],
                        mybir.ActivationFunctionType.Exp, scale=SCALE)
                    return S, E

                def emit_av_norm(s, ep, E):
                    msg = ps_m.tile([128, 2 * 195], F32, tag="msg",
                                    name=f"msg_{ep}")
                    for ni, (n0, nl) in enumerate(NHC):
                        for h in range(H):
                            nc.tensor.matmul(
                                msg[0:nl, ni * 195 + h * 65: ni * 195 + (h + 1) * 65],
                                E[0:128, h, n0: n0 + nl],
                                v_hi[0:128, s * 195 + h * 65: s * 195 + (h + 1) * 65],
                                start=True, stop=False)
                            nc.tensor.matmul(
                                msg[0:nl, ni * 195 + h * 65: ni * 195 + (h + 1) * 65],
                                E[0:70, h, N + n0: N + n0 + nl],
                                v_lo[0:70, s * 195 + h * 65: s * 195 + (h + 1) * 65],
                                start=False, stop=True)
                    rec = sbb.tile([128, 2, 3, 1], F32, tag="rec", name=f"rec_{ep}")
                    nc.vector.reciprocal(
                        rec[:],
                        msg[:].rearrange("p (i h c) -> p i h c", i=2, c=65)[:, :, :, 64:65])
                    for ni, (n0, nl) in enumerate(NHC):
                        for h in range(H):
                            a_sl = acc[0:nl,
                                       s * 2 * C + ni * C + h * 64:
                                       s * 2 * C + ni * C + (h + 1) * 64]
                            m_sl = msg[0:nl, ni * 195 + h * 65: ni * 195 + h * 65 + 64]
                            if h == 1:
                                # offload the middle head's scale to ScalarE
                                tmp = sbb.tile([128, 64], BF16, tag=f"tmp{ni}",
                                               name=f"tmp{ni}_{ep}")
                                nc.scalar.activation(
                                    tmp[0:nl, :], m_sl,
                                    mybir.ActivationFunctionType.Copy,
                                    scale=rec[0:nl, ni, h, :])
                                nc.vector.tensor_tensor(
                                    out=a_sl, in0=tmp[0:nl, :], in1=a_sl,
                                    op=mybir.AluOpType.add)
                            else:
                                nc.vector.scalar_tensor_tensor(
                                    out=a_sl, in0=m_sl,
                                    scalar=rec[0:nl, ni, h, :],
                                    in1=a_sl,
                                    op0=mybir.AluOpType.mult,
                                    op1=mybir.AluOpType.add)

                prev = None
                for (s, ep) in edges:
                    S, E = emit_qk_exp(s, ep)
                    if prev is not None:
                        emit_av_norm(*prev)
                    prev = (s, ep, E)
                emit_av_norm(*prev)

            # ---------------- phase C: proj + residual + LN2 stats ----------------
            with tc.tile_pool(name="pc_sb", bufs=3) as sbc, \
                 tc.tile_pool(name="ps_t", bufs=2, space="PSUM") as ps_t, \
                 tc.tile_pool(name="ps_c", bufs=2, space="PSUM") as ps_c:
                for s in range(SLOTS):
                    acc_bf = acc[:, s * 2 * C:(s + 1) * 2 * C]
                    xt2 = sbc.tile([128, 2, C], F32, tag="xt2")
                    nc.sync.dma_start(
                        xt2[:],
                        x_in[s * N: s * N + 256, :]
                        .rearrange("(i p) c -> p i c", p=128))
                    agT_a = sbc.tile([128, N], BF16, tag="agTa")
                    agT_b = sbc.tile([64, N], BF16, tag="agTb")
                    for ni, (n0, nl) in enumerate(NHC):
                        tp0 = ps_t.tile([128, 128], BF16, tag="tp0")
                        tp1 = ps_t.tile([64, 128], BF16, tag="tp1")
                        nc.tensor.transpose(tp0[:, 0:nl], acc_bf[0:nl, ni * C:ni * C + 128],
                                            ident_bf[0:nl, 0:nl])
                        nc.tensor.transpose(tp1[:, 0:nl], acc_bf[0:nl, ni * C + 128:ni * C + 192],
                                            ident_bf[0:nl, 0:nl])
                        nc.vector.tensor_copy(out=agT_a[:, n0:n0 + nl], in_=tp0[:, 0:nl])
                        nc.scalar.copy(out=agT_b[:, n0:n0 + nl], in_=tp1[:, 0:nl])
                    for ni, (n0, nl) in enumerate(NHC):
                        yp = ps_c.tile([128, C], F32, tag="yp")
                        nc.tensor.matmul(yp[0:nl, :], agT_a[:, n0:n0 + nl], pw_a[:],
                                         start=True, stop=False)
                        nc.tensor.matmul(yp[0:nl, :], agT_b[:, n0:n0 + nl], pw_b[:],
                                         start=False, stop=False)
                        nc.tensor.matmul(yp[0:nl, :],
                                         mrow[0:1, s * N + n0: s * N + n0 + nl],
                                         pb_sb[:], start=False, stop=True)
                        r0 = s * N + n0
                        x2t = sbc.tile([128, C], F32, tag="x2t")
                        nc.vector.tensor_tensor(out=x2t[0:nl, :], in0=yp[0:nl, :],
                                                in1=xt2[0:nl, ni, :],
                                                op=mybir.AluOpType.add)
                        nc.sync.dma_start(x2_dram[r0:r0 + nl, :], x2t[0:nl, :])
                        nc.gpsimd.tensor_copy(
                            out=x2_sb[0:nl, (s * 2 + ni) * C:(s * 2 + ni + 1) * C],
                            in_=x2t[0:nl, :])
                        st6 = sbc.tile([128, 6], F32, tag="st6")
                        nc.vector.bn_stats(st6[0:nl, :], x2t[0:nl, :])
                        nc.vector.bn_aggr(stats[0:nl, s * 2 + ni, :], st6[0:nl, :])

            nc.scalar.activation(sd[:], stats[:, :, 1:2],
                                 mybir.ActivationFunctionType.Sqrt,
                                 bias=eps_t[:])
            nc.vector.reciprocal(istd[:], sd[:])

            # ---------------- phase C2: xh2T build ----------------
            with tc.tile_pool(name="c2_sb", bufs=3) as sb2, \
                 tc.tile_pool(name="c2_ps", bufs=2, space="PSUM") as ps2:
                for s in range(SLOTS):
                    for ni, (n0, nl) in enumerate(NHC):
                        r0 = s * N + n0
                        xh2 = sb2.tile([128, C], BF16, tag="xh2")
                        nc.vector.tensor_scalar(xh2[0:nl, :],
                                                x2_sb[0:nl, (s * 2 + ni) * C:
                                                      (s * 2 + ni + 1) * C],
                                                stats[0:nl, s * 2 + ni, 0:1],
                                                istd[0:nl, s * 2 + ni, :],
                                                mybir.AluOpType.subtract,
                                                mybir.AluOpType.mult)
                        tp0 = ps2.tile([128, 128], BF16, tag="tp0")
                        tp1 = ps2.tile([64, 128], BF16, tag="tp1")
                        nc.tensor.transpose(tp0[:, 0:nl], xh2[0:nl, 0:128],
                                            ident_bf[0:nl, 0:nl])
                        nc.tensor.transpose(tp1[:, 0:nl], xh2[0:nl, 128:192],
                                            ident_bf[0:nl, 0:nl])
                        nc.vector.tensor_copy(out=xh2T_a[:, r0:r0 + nl], in_=tp0[:, 0:nl])
                        nc.scalar.copy(out=xh2T_b[0:64, r0:r0 + nl], in_=tp1[:, 0:nl])

            # ---------------- phase D: MLP ----------------
            with tc.tile_pool(name="pd_cst", bufs=1) as cd, \
                 tc.tile_pool(name="pd_sb", bufs=3) as sbd, \
                 tc.tile_pool(name="ps_d", bufs=2, space="PSUM") as ps_d:
                w1_a = cd.tile([128, HID], BF16)
                w1_b = cd.tile([65, HID], BF16)
                nc.sync.dma_start(w1_a[:], w1_in[0:128, :])
                nc.sync.dma_start(w1_b[:], w1_in[128:193, :])
                w2_t = []
                for hc in range(6):
                    t = cd.tile([128, C], BF16, tag=f"w2_{hc}", name=f"w2_{hc}")
                    nc.sync.dma_start(t[:], w2_in[hc * 128:(hc + 1) * 128, :])
                    w2_t.append(t)
                w2_bias = cd.tile([1, C], BF16)
                nc.sync.dma_start(w2_bias[:], w2_in[HID:HID + 1, :])
                ones_sb = cd.tile([1, 128], BF16)
                nc.vector.memset(ones_sb[:], 1.0)

                BLK = 512
                for b0 in range(0, TOK, BLK):
                    bl = min(BLK, TOK - b0)
                    h1 = [sbd.tile([128, BLK], BF16, tag=f"h1_{hc}", name=f"h1_{hc}_{b0}")
                          for hc in range(6)]
                    for hc in range(6):
                        hp = ps_d.tile([128, BLK], F32, tag="hp", name=f"hp_{hc}_{b0}")
                        nc.tensor.matmul(hp[:, 0:bl], w1_a[:, hc * 128:(hc + 1) * 128],
                                         xh2T_a[:, b0:b0 + bl], start=True, stop=False)
                        nc.tensor.matmul(hp[:, 0:bl], w1_b[:, hc * 128:(hc + 1) * 128],
                                         xh2T_b[:, b0:b0 + bl], start=False, stop=True)
                        nc.scalar.activation(h1[hc][:, 0:bl], hp[:, 0:bl],
                                             mybir.ActivationFunctionType.Gelu)
                    for u0 in range(0, bl, 256):
                        gu = b0 + u0
                        x2t = sbd.tile([128, 2, C], F32, tag="x2t",
                                       name=f"x2t_{gu}")
                        nc.sync.dma_start(
                            x2t[:],
                            x2_dram[gu:gu + 256, :]
                            .rearrange("(i p) c -> p i c", p=128))
                        ot = sbd.tile([128, 2, C], F32, tag="ot", name=f"ot_{gu}")
                        for v in range(2):
                            t0 = u0 + v * 128
                            if t0 >= bl:
                                continue
                            tl = min(128, bl - t0)
                            op = ps_d.tile([128, C], F32, tag="op",
                                           name=f"op_{b0}_{t0}")
                            for hc in range(6):
                                nc.tensor.matmul(op[0:tl, :], h1[hc][:, t0:t0 + tl],
                                                 w2_t[hc][:], start=(hc == 0),
                                                 stop=False)
                            nc.tensor.matmul(op[0:tl, :],
                                             ones_sb[0:1, 0:tl],
                                             w2_bias[:], start=False, stop=True)
                            nc.vector.tensor_tensor(out=ot[0:tl, v, :],
                                                    in0=op[0:tl, :],
                                                    in1=x2t[0:tl, v, :],
                                                    op=mybir.AluOpType.add)
                        rem = min(256, TOK - gu)
                        if rem == 256:
                            nc.sync.dma_start(
                                out[gu:gu + 256, :]
                                .rearrange("(i p) c -> p i c", p=128),
                                ot[:])
                        else:
                            nc.sync.dma_start(out[gu:gu + 128, :], ot[:, 0, :])
                            if rem > 128:
                                nc.sync.dma_start(out[gu + 128:gu + rem, :],
                                                  ot[0:rem - 128, 1, :])
    nc.compile()
    return nc


def kernel(x, egde, norm1_g, norm1_b, qkv_w, proj_w, proj_b,
           norm2_g, norm2_b, fc1_w, fc1_b, fc2_w, fc2_b):
    x = np.asarray(x, np.float32)
    edge = np.asarray(egde)
    g1 = np.asarray(norm1_g, np.float32)
    b1 = np.asarray(norm1_b, np.float32)
    qkv_w = np.asarray(qkv_w, np.float32)

    cores, prof, e_pad, starts, sched_src, is_pad, degs, mask = _plan(edge)

    # ---- kernel 1 host prep ----
    wqk = (qkv_w[0:2 * C, :] * g1[None, :]).T             # [C, 384]
    bqk = qkv_w[0:2 * C, :] @ b1
    wqkT_aug = _bf(np.concatenate([wqk, bqk[None, :]], 0))
    # v weights rearranged to (h, 65) with zero rowsum columns
    wv = (qkv_w[2 * C:3 * C, :] * g1[None, :]).T          # [C, 192]
    bv = qkv_w[2 * C:3 * C, :] @ b1                       # [192]
    wv_aug = np.zeros((C + 1, H * 65), np.float32)
    for h in range(H):
        wv_aug[0:C, h * 65:h * 65 + 64] = wv[:, h * 64:(h + 1) * 64]
        wv_aug[C, h * 65:h * 65 + 64] = bv[h * 64:(h + 1) * 64]
    wvT_aug = _bf(wv_aug)
    ones_row = _bf(np.ones((1, TOK), np.float32))

    x_own = np.stack([x[cores[c]].reshape(TOK, C) for c in range(NCORES)])

    nc1 = _build_kernel1()
    in_maps1 = [{"x_own": np.ascontiguousarray(x_own[c]),
                 "wqkT_aug": wqkT_aug, "wvT_aug": wvT_aug,
                 "ones_row": ones_row}
                for c in range(NCORES)]
    res1 = bass_utils.run_bass_kernel_spmd(nc1, in_maps1, core_ids=list(range(NCORES)),
                                           trace=TRACE)

    # ---- host gather: build per-edge k tables + augmented inputs ----
    # kT_tab: [64, (h, TOK)] -> global [64, H, Bn, N]
    kt_glob = np.zeros((HD, H, Bn, N), bf)
    for c in range(NCORES):
        sh = res1.results[c]["kT_tab"].reshape(HD, H, SLOTS, N)
        for s in range(SLOTS):
            kt_glob[:, :, cores[c][s], :] = sh[:, :, s, :]
    kte = np.zeros((NCORES, e_pad * (HD + 1), KCOLS), bf)
    beta_bf = np.array(BETA, bf)
    for c in range(NCORES):
        gathered = kt_glob[:, :, sched_src[c], :]         # [64, H, e_pad, N]
        blk = np.zeros((e_pad, HD + 1, H, KC), bf)
        blk[:, 0:HD, :, 0:N] = gathered.transpose(2, 0, 1, 3)
        # beta row: -1e5 on real m-cols of pad edges; 0 on fake/pad cols
        blk[is_pad[c], HD, :, 0:N] = beta_bf
        kte[c] = blk.reshape(e_pad * (HD + 1), KCOLS)

    # q tables + ones contract row
    qt_aug = np.zeros((NCORES, HD + 1, H * TOK), bf)
    for c in range(NCORES):
        qt_aug[c, 0:HD] = res1.results[c]["qT_tab"]
        qt_aug[c, HD] = np.array(1.0, bf)

    # v tables: deg-scaled rowsum columns + fake row (eps)
    v_hi_aug = np.zeros((NCORES, 128, SLOTS * 195), bf)
    v_lo_aug = np.zeros((NCORES, 70, SLOTS * 195), bf)
    fake = np.zeros((SLOTS * 195,), np.float32)
    for c in range(NCORES):
        vh = np.asarray(res1.results[c]["v_hi"], bf).copy()
        vl = np.asarray(res1.results[c]["v_lo"], bf).copy()
        for s in range(SLOTS):
            d = float(degs[c, s])
            for h in range(H):
                col = s * 195 + h * 65 + 64
                vh[:, col] = np.array(d, bf)
                vl[:, col] = np.array(d, bf)
                fake[col] = FAKE_EPS
        v_hi_aug[c, :, :] = vh
        v_lo_aug[c, 0:69, :] = vl
        v_lo_aug[c, 69, :] = fake.astype(bf)

    # ---- kernel 2 host prep ----
    g2 = np.asarray(norm2_g, np.float32)
    b2 = np.asarray(norm2_b, np.float32)
    fc1_w = np.asarray(fc1_w, np.float32)
    fc2_w = np.asarray(fc2_w, np.float32)
    w1 = (fc1_w * g2[None, :]).T
    bb1 = fc1_w @ b2 + np.asarray(fc1_b, np.float32)
    w1T_aug = _bf(np.concatenate([w1, bb1[None, :]], 0))
    w2T_aug = _bf(np.concatenate([fc2_w.T, np.asarray(fc2_b, np.float32)[None, :]], 0))
    projWT = _bf(np.asarray(proj_w, np.float32).T)
    projb = _bf(np.asarray(proj_b, np.float32)[None, :])
    maskrow = _bf(np.repeat(mask, N, axis=1)[:, None, :])

    x_own_pad = np.zeros((NCORES, TOK + 64, C), np.float32)
    x_own_pad[:, 0:TOK, :] = x_own

    nc2 = _build_kernel2(prof, e_pad)
    in_maps2 = []
    for c in range(NCORES):
        in_maps2.append({
            "x_own": np.ascontiguousarray(x_own_pad[c]),
            "qT_aug": np.ascontiguousarray(qt_aug[c]),
            "v_hi_aug": np.ascontiguousarray(v_hi_aug[c]),
            "v_lo_aug": np.ascontiguousarray(v_lo_aug[c]),
            "kT_edges": np.ascontiguousarray(kte[c]),
            "maskrow": np.ascontiguousarray(maskrow[c]),
            "ones_row": ones_row,
            "projWT": projWT, "projb": projb,
            "w1T_aug": w1T_aug, "w2T_aug": w2T_aug,
        })
    res2 = bass_utils.run_bass_kernel_spmd(nc2, in_maps2, core_ids=list(range(NCORES)),
                                           trace=TRACE)
    global LAST_EXEC_NS
    LAST_EXEC_NS = [res1.exec_time_ns or 0, res2.exec_time_ns or 0]

    outp = np.zeros((Bn, N, C), np.float32)
    for c in range(NCORES):
        outp[cores[c]] = res2.results[c]["out_own"].reshape(SLOTS, N, C)
    return outp


# revision 23
# speedup vs baseline: 1.2204x; 1.0096x over previous
"""Trainium2 Bass kernel for nn_Block_21792664060277 (gnn_message_passing).

Strategy (8 NeuronCores, SPMD):
  - Destination-node sharding: 256 graph nodes -> 8 cores x 32 slots,
    greedy-balanced by in-degree; scatter-mean becomes core-local.
  - Kernel 1 (per core, own 32 nodes): LayerNorm (stats via bn_stats,
    weights folded), qkv projection producing h-major transposed q/k
    tables [64, (h, tok)] and token-major v tables.
  - Host: gathers k tables into per-edge order, appends an augmented
    contract row (beta row: 0 for real edges, -1e5 for padding edges so
    exp underflows to exactly 0) and a fake m-column per head whose V
    row is (0..0, eps) so padded rowsums are eps (no NaN); scales the
    V ones-column by deg so the reciprocal of the rowsum directly gives
    the scatter-mean weight.
  - Kernel 2 (per core): per-edge QK^T (contract 65) -> ONE exp
    activation per edge over all heads/chunks -> AV with rowsum column
    -> reciprocal + 6 scalar_tensor_tensor accumulates; then proj
    (+masked bias), residual, LN2 + MLP (gelu), final residual.
"""
import sys

if "/opt/trn_rl_repo" not in sys.path:
    sys.path.insert(0, "/opt/trn_rl_repo")


def _ensure_ntff_hook():
    """Register the axon NTFF profiling hook if the antenv.axon_hooks
    shim module is absent (dropped from some images); without it
    run_bass_kernel_spmd(trace=True) silently skips profiling."""
    try:
        import antenv.axon_hooks  # noqa: F401
        return
    except ImportError:
        pass
    try:
        import types
        import antenv
        mod = types.ModuleType("antenv.axon_hooks")
        _h = {"hook": None}

        def set_axon_ntff_profile_hook(hook):
            _h["hook"] = hook

        def get_axon_ntff_profile_hook():
            return _h["hook"]

        mod.set_axon_ntff_profile_hook = set_axon_ntff_profile_hook
        mod.get_axon_ntff_profile_hook = get_axon_ntff_profile_hook
        sys.modules["antenv.axon_hooks"] = mod
        antenv.axon_hooks = mod
        from trn_agent_boot.trn_boot import _ntff_profile_via_ctypes
        hook = _ntff_profile_via_ctypes("/opt/axon/libaxon_pjrt.so")
        if hook is not None:
            set_axon_ntff_profile_hook(hook)
    except Exception:
        pass


_ensure_ntff_hook()

import numpy as np
import ml_dtypes

import concourse.bass as bass  # noqa: F401
import concourse.bacc as bacc
import concourse.mybir as mybir
import concourse.tile as tile
from concourse import bass_utils
from concourse.masks import make_identity

BF16 = mybir.dt.bfloat16
F32 = mybir.dt.float32

Bn, N, C = 256, 197, 192
H, HD = 3, 64
HID = 768
NCORES = 8
SLOTS = 32
TOK = SLOTS * N          # 6304
EPS = 1e-5
SCALE = HD ** -0.5
BETA = -1.0e5            # pad-edge logit offset (exp underflows to 0)
FAKE_EPS = 1e-30         # fake-column rowsum epsilon
KC = 198                 # per-head k-table cols (197 real + 1 fake)
KCOLS = H * KC           # 594: per-edge k table cols (h-major)

MC = [(0, 128), (128, 69)]      # token chunks within a node (197 = 128+69)

TRACE = False
LAST_EXEC_NS = None

bf = ml_dtypes.bfloat16


def _bf(a):
    return np.ascontiguousarray(np.asarray(a, np.float32)).astype(bf)


def _plan(edge):
    """Node->core assignment balanced by degree + shared degree profile."""
    src, dst = np.asarray(edge[0]), np.asarray(edge[1])
    deg = np.bincount(dst, minlength=Bn)
    order = np.argsort(-deg, kind="stable")
    cores = [[] for _ in range(NCORES)]
    loads = np.zeros(NCORES, np.int64)
    for n in order:
        free = [c for c in range(NCORES) if len(cores[c]) < SLOTS]
        c = min(free, key=lambda c: (loads[c], c))
        cores[c].append(int(n))
        loads[c] += deg[n]
    prof = np.zeros(SLOTS, np.int64)
    for c in range(NCORES):
        ds = np.array([deg[n] for n in cores[c]])
        prof = np.maximum(prof, ds)
    prof = prof.astype(int)
    e_pad = max(int(prof.sum()), 1)
    starts = np.concatenate([[0], np.cumsum(prof)]).astype(int)
    by_dst = [[] for _ in range(Bn)]
    for e in range(src.shape[0]):
        by_dst[int(dst[e])].append(int(src[e]))
    sched_src = np.zeros((NCORES, e_pad), np.int64)
    is_pad = np.ones((NCORES, e_pad), bool)
    degs = np.zeros((NCORES, SLOTS), np.int64)
    mask = np.zeros((NCORES, SLOTS), np.float32)
    for c in range(NCORES):
        for s in range(SLOTS):
            node = cores[c][s]
            lst = by_dst[node]
            degs[c, s] = len(lst)
            mask[c, s] = 1.0 if lst else 0.0
            for j in range(prof[s]):
                p = starts[s] + j
                if j < len(lst):
                    sched_src[c, p] = lst[j]
                    is_pad[c, p] = False
    return cores, prof, e_pad, starts, sched_src, is_pad, degs, mask


def _build_kernel1():
    nc = bacc.Bacc("TRN2", target_bir_lowering=False, debug=False,
                   num_devices=NCORES)
    x_in = nc.dram_tensor("x_own", [TOK, C], F32, kind="ExternalInput")
    wqk = nc.dram_tensor("wqkT_aug", [C + 1, 2 * C], BF16, kind="ExternalInput")
    wv = nc.dram_tensor("wvT_aug", [C + 1, H * 65], BF16, kind="ExternalInput")
    ones_in = nc.dram_tensor("ones_row", [1, TOK], BF16, kind="ExternalInput")
    qt_out = nc.dram_tensor("qT_tab", [HD, H * TOK], BF16, kind="ExternalOutput")
    kt_out = nc.dram_tensor("kT_tab", [HD, H * TOK], BF16, kind="ExternalOutput")
    v_hi_out = nc.dram_tensor("v_hi", [128, SLOTS * 195], BF16, kind="ExternalOutput")
    v_lo_out = nc.dram_tensor("v_lo", [69, SLOTS * 195], BF16, kind="ExternalOutput")

    NT = (TOK + 127) // 128      # 50 token tiles (49 full + 32)

    with tile.TileContext(nc) as tc:
        with tc.tile_pool(name="cst", bufs=1) as cst:
            ident = cst.tile([128, 128], BF16)
            make_identity(nc, ident[:])
            eps_t = cst.tile([128, 1], F32)
            nc.vector.memset(eps_t[:], EPS)
            wqk_a = cst.tile([128, 2 * C], BF16)
            wqk_b = cst.tile([65, 2 * C], BF16)
            nc.sync.dma_start(wqk_a[:], wqk[0:128, :])
            nc.sync.dma_start(wqk_b[:], wqk[128:193, :])
            wv_a = cst.tile([128, H * 65], BF16)
            wv_b = cst.tile([65, H * 65], BF16)
            nc.sync.dma_start(wv_a[:], wv[0:128, :])
            nc.sync.dma_start(wv_b[:], wv[128:193, :])
            x_res = cst.tile([128, NT * C], F32)
            stats = cst.tile([128, NT, 2], F32)
            sd = cst.tile([128, NT, 1], F32)
            istd = cst.tile([128, NT, 1], F32)
            xhT_a = cst.tile([128, TOK], BF16)
            xhT_b = cst.tile([65, TOK], BF16)
            nc.sync.dma_start(xhT_b[64:65, :], ones_in[:])

            TGRP = 4
            with tc.tile_pool(name="pa", bufs=3) as sba, \
                 tc.tile_pool(name="pbt", bufs=1, space="PSUM") as pbt, \
                 tc.tile_pool(name="pc", bufs=3) as sbc, \
                 tc.tile_pool(name="pcp", bufs=2, space="PSUM") as pcp, \
                 tc.tile_pool(name="pd", bufs=3) as sbd, \
                 tc.tile_pool(name="pdp", bufs=2, space="PSUM") as pdp:
                # ---- pass A: load x (batched), LN stats ----
                for g0 in range(0, TOK, 512):
                    gl = min(512, TOK - g0)
                    if gl == 512:
                        nc.sync.dma_start(
                            x_res[:, g0 // 128 * C:(g0 // 128 + 4) * C]
                            .rearrange("p (i c) -> p i c", c=C),
                            x_in[g0:g0 + 512, :].rearrange("(i p) c -> p i c", p=128))
                    else:
                        for t0 in range(g0, TOK, 128):
                            tl = min(128, TOK - t0)
                            nc.sync.dma_start(
                                x_res[0:tl, t0 // 128 * C:(t0 // 128 + 1) * C],
                                x_in[t0:t0 + tl, :])
                for t in range(NT):
                    tl = min(128, TOK - t * 128)
                    st6 = sba.tile([128, 6], F32, tag="st6")
                    nc.vector.bn_stats(st6[0:tl, :], x_res[0:tl, t * C:(t + 1) * C])
                    nc.vector.bn_aggr(stats[0:tl, t, :], st6[0:tl, :])
                nc.scalar.activation(sd[:], stats[:, :, 1:2],
                                     mybir.ActivationFunctionType.Sqrt,
                                     bias=eps_t[:])
                nc.vector.reciprocal(istd[:], sd[:])

                # ---- pass B: xhat + transpose ----
                for t in range(NT):
                    g0 = t * 128
                    tl = min(128, TOK - g0)
                    xh = sba.tile([128, C], BF16, tag="xh")
                    nc.vector.tensor_scalar(xh[0:tl, :], x_res[0:tl, t * C:(t + 1) * C],
                                            stats[0:tl, t, 0:1], istd[0:tl, t, :],
                                            mybir.AluOpType.subtract,
                                            mybir.AluOpType.mult)
                    tp0 = pbt.tile([128, 128], BF16, tag="tp0")
                    tp1 = pbt.tile([64, 128], BF16, tag="tp1")
                    nc.tensor.transpose(tp0[:, 0:tl], xh[0:tl, 0:128], ident[0:tl, 0:tl])
                    nc.tensor.transpose(tp1[:, 0:tl], xh[0:tl, 128:192], ident[0:tl, 0:tl])
                    nc.vector.tensor_copy(out=xhT_a[:, g0:g0 + tl], in_=tp0[:, 0:tl])
                    nc.scalar.copy(out=xhT_b[0:64, g0:g0 + tl], in_=tp1[:, 0:tl])

                # ---- pass C: q/k projections (h-major tables) ----
                for gg in range(0, NT, TGRP):
                    gn = min(TGRP, NT - gg)
                    for cc in range(3):
                        qkp = pcp.tile([128, TGRP * 128], F32, tag="qkp",
                                       name=f"qkp_{gg}_{cc}")
                        for tt in range(gn):
                            g0 = (gg + tt) * 128
                            tl = min(128, TOK - g0)
                            nc.tensor.matmul(qkp[:, tt * 128:tt * 128 + tl],
                                             wqk_a[:, cc * 128:(cc + 1) * 128],
                                             xhT_a[:, g0:g0 + tl],
                                             start=True, stop=False)
                            nc.tensor.matmul(qkp[:, tt * 128:tt * 128 + tl],
                                             wqk_b[:, cc * 128:(cc + 1) * 128],
                                             xhT_b[:, g0:g0 + tl],
                                             start=False, stop=True)
                        g0 = gg * 128
                        glen = min(TGRP * 128, TOK - g0)
                        for half in range(2):
                            gidx = cc * 2 + half
                            if gidx < 3:
                                dstt, hh = qt_out, gidx
                            else:
                                dstt, hh = kt_out, gidx - 3
                            stg = sbc.tile([64, TGRP * 128], BF16, tag="stg")
                            if half == 0:
                                nc.vector.tensor_copy(out=stg[:, 0:glen],
                                                      in_=qkp[0:64, 0:glen])
                            else:
                                nc.scalar.copy(out=stg[:, 0:glen],
                                               in_=qkp[64:128, 0:glen])
                            nc.sync.dma_start(
                                dstt[:, hh * TOK + g0: hh * TOK + g0 + glen],
                                stg[:, 0:glen])

                # ---- pass D: v projection (token-major per slot) ----
                for s in range(SLOTS):
                    for mi, (m0, ml) in enumerate(MC):
                        r0 = s * N + m0
                        vp = pdp.tile([128, H * 65], F32, tag="vp")
                        nc.tensor.matmul(vp[0:ml, :], xhT_a[:, r0:r0 + ml], wv_a[:],
                                         start=True, stop=False)
                        nc.tensor.matmul(vp[0:ml, :], xhT_b[:, r0:r0 + ml], wv_b[:],
                                         start=False, stop=True)
                        vsb = sbd.tile([128, H * 65], BF16, tag="vsb")
                        if mi == 0:
                            nc.vector.tensor_copy(out=vsb[0:ml, :], in_=vp[0:ml, :])
                        else:
                            nc.scalar.copy(out=vsb[0:ml, :], in_=vp[0:ml, :])
                        dstt = v_hi_out if mi == 0 else v_lo_out
                        nc.sync.dma_start(dstt[0:ml, s * 195:(s + 1) * 195],
                                          vsb[0:ml, :])
    nc.compile()
    return nc


def _build_kernel2(prof, e_pad):
    starts = np.concatenate([[0], np.cumsum(prof)]).astype(int)
    nc = bacc.Bacc("TRN2", target_bir_lowering=False, debug=False,
                   num_devices=NCORES)
    x_in = nc.dram_tensor("x_own", [TOK + 64, C], F32, kind="ExternalInput")
    qt_in = nc.dram_tensor("qT_aug", [HD + 1, H * TOK], BF16, kind="ExternalInput")
    v_hi_in = nc.dram_tensor("v_hi_aug", [128, SLOTS * 195], BF16, kind="ExternalInput")
    v_lo_in = nc.dram_tensor("v_lo_aug", [70, SLOTS * 195], BF16, kind="ExternalInput")
    kte_in = nc.dram_tensor("kT_edges", [e_pad * (HD + 1), KCOLS], BF16,
                            kind="ExternalInput")
    mrow_in = nc.dram_tensor("maskrow", [1, TOK], BF16, kind="ExternalInput")
    ones_in = nc.dram_tensor("ones_row", [1, TOK], BF16, kind="ExternalInput")
    pw_in = nc.dram_tensor("projWT", [C, C], BF16, kind="ExternalInput")
    pb_in = nc.dram_tensor("projb", [1, C], BF16, kind="ExternalInput")
    w1_in = nc.dram_tensor("w1T_aug", [C + 1, HID], BF16, kind="ExternalInput")
    w2_in = nc.dram_tensor("w2T_aug", [HID + 1, C], BF16, kind="ExternalInput")
    out = nc.dram_tensor("out_own", [TOK, C], F32, kind="ExternalOutput")

    NHC = [(0, 128), (128, 69)]   # n-chunks (dest tokens) within a node
    NT = (TOK + 127) // 128

    with tile.TileContext(nc) as tc:
        with tc.tile_pool(name="cst", bufs=1) as cst, \
             tc.tile_pool(name="dram", bufs=1, space="DRAM") as dpool:
            x2_dram = dpool.tile([TOK + 96, C], F32)
            ident_bf = cst.tile([128, 128], BF16)
            make_identity(nc, ident_bf[:])
            eps_t = cst.tile([128, 1], F32)
            nc.vector.memset(eps_t[:], EPS)
            qt_sb = cst.tile([HD + 1, H * TOK], BF16)
            nc.sync.dma_start(qt_sb[:], qt_in[:])
            v_hi = cst.tile([128, SLOTS * 195], BF16)
            v_lo = cst.tile([70, SLOTS * 195], BF16)
            nc.sync.dma_start(v_hi[:], v_hi_in[:])
            nc.sync.dma_start(v_lo[:], v_lo_in[:])
            mrow = cst.tile([1, TOK], BF16)
            nc.sync.dma_start(mrow[:], mrow_in[:])
            pw_a = cst.tile([128, C], BF16)
            pw_b = cst.tile([64, C], BF16)
            nc.sync.dma_start(pw_a[:], pw_in[0:128, :])
            nc.sync.dma_start(pw_b[:], pw_in[128:192, :])
            pb_sb = cst.tile([1, C], BF16)
            nc.sync.dma_start(pb_sb[:], pb_in[:])
            acc = cst.tile([128, SLOTS * 2 * C], BF16)
            nc.vector.memset(acc[:], 0.0)
            x2_sb = cst.tile([128, SLOTS * 2 * C], BF16)
            stats = cst.tile([128, 2 * SLOTS, 2], F32)
            sd = cst.tile([128, 2 * SLOTS, 1], F32)
            istd = cst.tile([128, 2 * SLOTS, 1], F32)
            xh2T_a = cst.tile([128, TOK], BF16)
            xh2T_b = cst.tile([65, TOK], BF16)
            nc.sync.dma_start(xh2T_b[64:65, :], ones_in[:])

            # ---------------- phase B: per-edge attention ----------------
            edges = [(s, starts[s] + j) for s in range(SLOTS)
                     for j in range(int(prof[s]))]
            with tc.tile_pool(name="pb_sb", bufs=3) as sbb, \
                 tc.tile_pool(name="ps_s", bufs=2, space="PSUM") as ps_s, \
                 tc.tile_pool(name="ps_m", bufs=2, space="PSUM") as ps_m:

                def emit_qk_exp(s, ep):
                    kst = sbb.tile([HD + 1, KCOLS], BF16, tag="kst",
                                   name=f"kst_{ep}")
                    nc.sync.dma_start(kst[:], kte_in[ep * 65:(ep + 1) * 65, :])
                    S = ps_s.tile([128, 3 * 512], F32, tag="S", name=f"S_{ep}")
                    for h in range(H):
                        nc.tensor.matmul(
                            S[0:128, h * 512: h * 512 + N],
                            kst[:, h * KC: h * KC + 128],
                            qt_sb[:, h * TOK + s * N: h * TOK + (s + 1) * N],
                            start=True, stop=True)
                        nc.tensor.matmul(
                            S[0:70, h * 512 + N: h * 512 + 2 * N],
                            kst[:, h * KC + 128: h * KC + KC],
                            qt_sb[:, h * TOK + s * N: h * TOK + (s + 1) * N],
                            start=True, stop=True)
                    E = sbb.tile([128, 3, 2 * N], BF16, tag="E", name=f"E_{ep}")
                    nc.scalar.activation(
                        E[:],
                        S[:].rearrange("p (h c) -> p h c", h=3)[:, :, 0:2 * N],
                        mybir.ActivationFunctionType.Exp, scale=SCALE)
                    return S, E

                def emit_av_norm(s, ep, E):
                    msg = ps_m.tile([128, 2 * 195], F32, tag="msg",
                                    name=f"msg_{ep}")
                    for ni, (n0, nl) in enumerate(NHC):
                        for h in range(H):
                            nc.tensor.matmul(
                                msg[0:nl, ni * 195 + h * 65: ni * 195 + (h + 1) * 65],
                                E[0:128, h, n0: n0 + nl],
                                v_hi[0:128, s * 195 + h * 65: s * 195 + (h + 1) * 65],
                                start=True, stop=False)
                            nc.tensor.matmul(
                                msg[0:nl, ni * 195 + h * 65: ni * 195 + (h + 1) * 65],
                                E[0:70, h, N + n0: N + n0 + nl],
                                v_lo[0:70, s * 195 + h * 65: s * 195 + (h + 1) * 65],
                                start=False, stop=True)
                    rec = sbb.tile([128, 2, 3, 1], F32, tag="rec", name=f"rec_{ep}")
                    nc.vector.reciprocal(
                        rec[:],
                        msg[:].rearrange("p (i h c) -> p i h c", i=2, c=65)[:, :, :, 64:65])
                    for ni, (n0, nl) in enumerate(NHC):
                        for h in range(H):
                            a_sl = acc[0:nl,
                                       s * 2 * C + ni * C + h * 64:
                                       s * 2 * C + ni * C + (h + 1) * 64]
                            m_sl = msg[0:nl, ni * 195 + h * 65: ni * 195 + h * 65 + 64]
                            nc.vector.scalar_tensor_tensor(
                                out=a_sl, in0=m_sl,
                                scalar=rec[0:nl, ni, h, :],
                                in1=a_sl,
                                op0=mybir.AluOpType.mult,
                                op1=mybir.AluOpType.add)

                prev = None
                for (s, ep) in edges:
                    S, E = emit_qk_exp(s, ep)
                    if prev is not None:
                        emit_av_norm(*prev)
                    prev = (s, ep, E)
                emit_av_norm(*prev)

            # ---------------- phase C: proj + residual + LN2 stats ----------------
            with tc.tile_pool(name="pc_sb", bufs=3) as sbc, \
                 tc.tile_pool(name="ps_t", bufs=2, space="PSUM") as ps_t, \
                 tc.tile_pool(name="ps_c", bufs=2, space="PSUM") as ps_c:
                for s in range(SLOTS):
                    acc_bf = acc[:, s * 2 * C:(s + 1) * 2 * C]
                    xt2 = sbc.tile([128, 2, C], F32, tag="xt2")
                    nc.sync.dma_start(
                        xt2[:],
                        x_in[s * N: s * N + 256, :]
                        .rearrange("(i p) c -> p i c", p=128))
                    agT_a = sbc.tile([128, N], BF16, tag="agTa")
                    agT_b = sbc.tile([64, N], BF16, tag="agTb")
                    for ni, (n0, nl) in enumerate(NHC):
                        tp0 = ps_t.tile([128, 128], BF16, tag="tp0")
                        tp1 = ps_t.tile([64, 128], BF16, tag="tp1")
                        nc.tensor.transpose(tp0[:, 0:nl], acc_bf[0:nl, ni * C:ni * C + 128],
                                            ident_bf[0:nl, 0:nl])
                        nc.tensor.transpose(tp1[:, 0:nl], acc_bf[0:nl, ni * C + 128:ni * C + 192],
                                            ident_bf[0:nl, 0:nl])
                        nc.vector.tensor_copy(out=agT_a[:, n0:n0 + nl], in_=tp0[:, 0:nl])
                        nc.scalar.copy(out=agT_b[:, n0:n0 + nl], in_=tp1[:, 0:nl])
                    for ni, (n0, nl) in enumerate(NHC):
                        yp = ps_c.tile([128, C], F32, tag="yp")
                        nc.tensor.matmul(yp[0:nl, :], agT_a[:, n0:n0 + nl], pw_a[:],
                                         start=True, stop=False)
                        nc.tensor.matmul(yp[0:nl, :], agT_b[:, n0:n0 + nl], pw_b[:],
                                         start=False, stop=False)
                        nc.tensor.matmul(yp[0:nl, :],
                                         mrow[0:1, s * N + n0: s * N + n0 + nl],
                                         pb_sb[:], start=False, stop=True)
                        r0 = s * N + n0
                        x2t = sbc.tile([128, C], F32, tag="x2t")
                        nc.vector.tensor_tensor(out=x2t[0:nl, :], in0=yp[0:nl, :],
                                                in1=xt2[0:nl, ni, :],
                                                op=mybir.AluOpType.add)
                        nc.sync.dma_start(x2_dram[r0:r0 + nl, :], x2t[0:nl, :])
                        nc.gpsimd.tensor_copy(
                            out=x2_sb[0:nl, (s * 2 + ni) * C:(s * 2 + ni + 1) * C],
                            in_=x2t[0:nl, :])
                        st6 = sbc.tile([128, 6], F32, tag="st6")
                        nc.vector.bn_stats(st6[0:nl, :], x2t[0:nl, :])
                        nc.vector.bn_aggr(stats[0:nl, s * 2 + ni, :], st6[0:nl, :])

            nc.scalar.activation(sd[:], stats[:, :, 1:2],
                                 mybir.ActivationFunctionType.Sqrt,
                                 bias=eps_t[:])
            nc.vector.reciprocal(istd[:], sd[:])

            # ---------------- phase C2: xh2T build ----------------
            with tc.tile_pool(name="c2_sb", bufs=3) as sb2, \
                 tc.tile_pool(name="c2_ps", bufs=2, space="PSUM") as ps2:
                for s in range(SLOTS):
                    for ni, (n0, nl) in enumerate(NHC):
                        r0 = s * N + n0
                        xh2 = sb2.tile([128, C], BF16, tag="xh2")
                        nc.vector.tensor_scalar(xh2[0:nl, :],
                                                x2_sb[0:nl, (s * 2 + ni) * C:
                                                      (s * 2 + ni + 1) * C],
                                                stats[0:nl, s * 2 + ni, 0:1],
                                                istd[0:nl, s * 2 + ni, :],
                                                mybir.AluOpType.subtract,
                                                mybir.AluOpType.mult)
                        tp0 = ps2.tile([128, 128], BF16, tag="tp0")
                        tp1 = ps2.tile([64, 128], BF16, tag="tp1")
                        nc.tensor.transpose(tp0[:, 0:nl], xh2[0:nl, 0:128],
                                            ident_bf[0:nl, 0:nl])
                        nc.tensor.transpose(tp1[:, 0:nl], xh2[0:nl, 128:192],
                                            ident_bf[0:nl, 0:nl])
                        nc.vector.tensor_copy(out=xh2T_a[:, r0:r0 + nl], in_=tp0[:, 0:nl])
                        nc.scalar.copy(out=xh2T_b[0:64, r0:r0 + nl], in_=tp1[:, 0:nl])

            # ---------------- phase D: MLP ----------------
            with tc.tile_pool(name="pd_cst", bufs=1) as cd, \
                 tc.tile_pool(name="pd_sb", bufs=3) as sbd, \
                 tc.tile_pool(name="ps_d", bufs=2, space="PSUM") as ps_d:
                w1_a = cd.tile([128, HID], BF16)
                w1_b = cd.tile([65, HID], BF16)
                nc.sync.dma_start(w1_a[:], w1_in[0:128, :])
                nc.sync.dma_start(w1_b[:], w1_in[128:193, :])
                w2_t = []
                for hc in range(6):
                    t = cd.tile([128, C], BF16, tag=f"w2_{hc}", name=f"w2_{hc}")
                    nc.sync.dma_start(t[:], w2_in[hc * 128:(hc + 1) * 128, :])
                    w2_t.append(t)
                w2_bias = cd.tile([1, C], BF16)
                nc.sync.dma_start(w2_bias[:], w2_in[HID:HID + 1, :])
                ones_sb = cd.tile([1, 128], BF16)
                nc.vector.memset(ones_sb[:], 1.0)

                BLK = 512
                for b0 in range(0, TOK, BLK):
                    bl = min(BLK, TOK - b0)
                    h1 = [sbd.tile([128, BLK], BF16, tag=f"h1_{hc}", name=f"h1_{hc}_{b0}")
                          for hc in range(6)]
                    for hc in range(6):
                        hp = ps_d.tile([128, BLK], F32, tag="hp", name=f"hp_{hc}_{b0}")
                        nc.tensor.matmul(hp[:, 0:bl], w1_a[:, hc * 128:(hc + 1) * 128],
                                         xh2T_a[:, b0:b0 + bl], start=True, stop=False)
                        nc.tensor.matmul(hp[:, 0:bl], w1_b[:, hc * 128:(hc + 1) * 128],
                                         xh2T_b[:, b0:b0 + bl], start=False, stop=True)
                        nc.scalar.activation(h1[hc][:, 0:bl], hp[:, 0:bl],
                                             mybir.ActivationFunctionType.Gelu)
                    for u0 in range(0, bl, 256):
                        gu = b0 + u0
                        x2t = sbd.tile([128, 2, C], F32, tag="x2t",
                                       name=f"x2t_{gu}")
                        nc.sync.dma_start(
                            x2t[:],
                            x2_dram[gu:gu + 256, :]
                            .rearrange("(i p) c -> p i c", p=128))
                        ot = sbd.tile([128, 2, C], F32, tag="ot", name=f"ot_{gu}")
                        for v in range(2):
                            t0 = u0 + v * 128
                            if t0 >= bl:
                                continue
                            tl = min(128, bl - t0)
                            op = ps_d.tile([128, C], F32, tag="op",
                                           name=f"op_{b0}_{t0}")
                            for hc in range(6):
                                nc.tensor.matmul(op[0:tl, :], h1[hc][:, t0:t0 + tl],
                                                 w2_t[hc][:], start=(hc == 0),
                                                 stop=False)
                            nc.tensor.matmul(op[0:tl, :],
                                             ones_sb[0:1, 0:tl],
                                             w2_bias[:], start=False, stop=True)
                            nc.vector.tensor_tensor(out=ot[0:tl, v, :],
                                                    in0=op[0:tl, :],
                                                    in1=x2t[0:tl, v, :],
                                                    op=mybir.AluOpType.add)
                        rem = min(256, TOK - gu)
                        if rem == 256:
                            nc.sync.dma_start(
                                out[gu:gu + 256, :]
                                .rearrange("(i p) c -> p i c", p=128),
                                ot[:])
                        else:
                            nc.sync.dma_start(out[gu:gu + 128, :], ot[:, 0, :])
                            if rem > 128:
                                nc.sync.dma_start(out[gu + 128:gu + rem, :],
                                                  ot[0:rem - 128, 1, :])
    nc.compile()
    return nc


def kernel(x, egde, norm1_g, norm1_b, qkv_w, proj_w, proj_b,
           norm2_g, norm2_b, fc1_w, fc1_b, fc2_w, fc2_b):
    x = np.asarray(x, np.float32)
    edge = np.asarray(egde)
    g1 = np.asarray(norm1_g, np.float32)
    b1 = np.asarray(norm1_b, np.float32)
    qkv_w = np.asarray(qkv_w, np.float32)

    cores, prof, e_pad, starts, sched_src, is_pad, degs, mask = _plan(edge)

    # ---- kernel 1 host prep ----
    wqk = (qkv_w[0:2 * C, :] * g1[None, :]).T             # [C, 384]
    bqk = qkv_w[0:2 * C, :] @ b1
    wqkT_aug = _bf(np.concatenate([wqk, bqk[None, :]], 0))
    # v weights rearranged to (h, 65) with zero rowsum columns
    wv = (qkv_w[2 * C:3 * C, :] * g1[None, :]).T          # [C, 192]
    bv = qkv_w[2 * C:3 * C, :] @ b1                       # [192]
    wv_aug = np.zeros((C + 1, H * 65), np.float32)
    for h in range(H):
        wv_aug[0:C, h * 65:h * 65 + 64] = wv[:, h * 64:(h + 1) * 64]
        wv_aug[C, h * 65:h * 65 + 64] = bv[h * 64:(h + 1) * 64]
    wvT_aug = _bf(wv_aug)
    ones_row = _bf(np.ones((1, TOK), np.float32))

    x_own = np.stack([x[cores[c]].reshape(TOK, C) for c in range(NCORES)])

    nc1 = _build_kernel1()
    in_maps1 = [{"x_own": np.ascontiguousarray(x_own[c]),
                 "wqkT_aug": wqkT_aug, "wvT_aug": wvT_aug,
                 "ones_row": ones_row}
                for c in range(NCORES)]
    res1 = bass_utils.run_bass_kernel_spmd(nc1, in_maps1, core_ids=list(range(NCORES)),
                                           trace=TRACE)

    # ---- host gather: build per-edge k tables + augmented inputs ----
    # kT_tab: [64, (h, TOK)] -> global [64, H, Bn, N]
    kt_glob = np.zeros((HD, H, Bn, N), bf)
    for c in range(NCORES):
        sh = res1.results[c]["kT_tab"].reshape(HD, H, SLOTS, N)
        for s in range(SLOTS):
            kt_glob[:, :, cores[c][s], :] = sh[:, :, s, :]
    kte = np.zeros((NCORES, e_pad * (HD + 1), KCOLS), bf)
    beta_bf = np.array(BETA, bf)
    for c in range(NCORES):
        gathered = kt_glob[:, :, sched_src[c], :]         # [64, H, e_pad, N]
        blk = np.zeros((e_pad, HD + 1, H, KC), bf)
        blk[:, 0:HD, :, 0:N] = gathered.transpose(2, 0, 1, 3)
        # beta row: -1e5 on real m-cols of pad edges; 0 on fake/pad cols
        blk[is_pad[c], HD, :, 0:N] = beta_bf
        kte[c] = blk.reshape(e_pad * (HD + 1), KCOLS)

    # q tables + ones contract row
    qt_aug = np.zeros((NCORES, HD + 1, H * TOK), bf)
    for c in range(NCORES):
        qt_aug[c, 0:HD] = res1.results[c]["qT_tab"]
        qt_aug[c, HD] = np.array(1.0, bf)

    # v tables: deg-scaled rowsum columns + fake row (eps)
    v_hi_aug = np.zeros((NCORES, 128, SLOTS * 195), bf)
    v_lo_aug = np.zeros((NCORES, 70, SLOTS * 195), bf)
    fake = np.zeros((SLOTS * 195,), np.float32)
    for c in range(NCORES):
        vh = np.asarray(res1.results[c]["v_hi"], bf).copy()
        vl = np.asarray(res1.results[c]["v_lo"], bf).copy()
        for s in range(SLOTS):
            d = float(degs[c, s])
            for h in range(H):
                col = s * 195 + h * 65 + 64
                vh[:, col] = np.array(d, bf)
                vl[:, col] = np.array(d, bf)
                fake[col] = FAKE_EPS
        v_hi_aug[c, :, :] = vh
        v_lo_aug[c, 0:69, :] = vl
        v_lo_aug[c, 69, :] = fake.astype(bf)

    # ---- kernel 2 host prep ----
    g2 = np.asarray(norm2_g, np.float32)
    b2 = np.asarray(norm2_b, np.float32)
    fc1_w = np.asarray(fc1_w, np.float32)
    fc2_w = np.asarray(fc2_w, np.float32)
    w1 = (fc1_w * g2[None, :]).T
    bb1 = fc1_w @ b2 + np.asarray(fc1_b, np.float32)
    w1T_aug = _bf(np.concatenate([w1, bb1[None, :]], 0))
    w2T_aug = _bf(np.concatenate([fc2_w.T, np.asarray(fc2_b, np.float32)[None, :]], 0))
    projWT = _bf(np.asarray(proj_w, np.float32).T)
    projb = _bf(np.asarray(proj_b, np.float32)[None, :])
    maskrow = _bf(np.repeat(mask, N, axis=1)[:, None, :])

    x_own_pad = np.zeros((NCORES, TOK + 64, C), np.float32)
    x_own_pad[:, 0:TOK, :] = x_own

    nc2 = _build_kernel2(prof, e_pad)
    in_maps2 = []
    for c in range(NCORES):
        in_maps2.append({
            "x_own": np.ascontiguousarray(x_own_pad[c]),
            "qT_aug": np.ascontiguousarray(qt_aug[c]),
            "v_hi_aug": np.ascontiguousarray(v_hi_aug[c]),
            "v_lo_aug": np.ascontiguousarray(v_lo_aug[c]),
            "kT_edges": np.ascontiguousarray(kte[c]),
            "maskrow": np.ascontiguousarray(maskrow[c]),
            "ones_row": ones_row,
            "projWT": projWT, "projb": projb,
            "w1T_aug": w1T_aug, "w2T_aug": w2T_aug,
        })
    res2 = bass_utils.run_bass_kernel_spmd(nc2, in_maps2, core_ids=list(range(NCORES)),
                                           trace=TRACE)
    global LAST_EXEC_NS
    LAST_EXEC_NS = [res1.exec_time_ns or 0, res2.exec_time_ns or 0]

    outp = np.zeros((Bn, N, C), np.float32)
    for c in range(NCORES):
        outp[cores[c]] = res2.results[c]["out_own"].reshape(SLOTS, N, C)
    return outp
